# revision 14
# baseline (speedup 1.0000x reference)
"""Trainium2 Bass kernel for nn_DecoderHead (MAE-style decoder head).

Strategy (8 NeuronCores): data-parallel over batch B=4 x 2-way token split
per batch. Cores sharing a batch both compute layer 0 for all 2048 tokens
(cheaper than cross-core K/V exchange), then layer 1 + head for their own
1024-token half only. Tokens are permuted host-side so every core's "own"
half sits at positions 1024..2048 -- attention is permutation-equivariant,
so one NEFF serves all cores.

On-device layout is feature-major (x^T: [D, T] with D on partitions), which
makes every linear a plain lhsT.T @ rhs chain with host-pre-transposed
weights and no activation transposes. Heads are padded 96->128 so per-head
Q/K/V slices are partition-aligned; a ones-row injected into V (via the
padded bias) makes the PV matmul emit softmax denominators for free.
Softmax skips max-subtraction (|scores| <= ~2 by construction). LN
gamma/beta are folded into the adjacent weights host-side; LN stats are
computed with ones-vector matmuls on the PE and broadcast across partitions
with rank-1 matmuls.
"""

import sys
import numpy as np

sys.path.insert(0, "/opt/trn_rl_repo")

import ml_dtypes

P = 128
B = 4
N_VIS = 512
T = 2048          # N_TOT
D = 768
KD = D // P       # 6
NH = 8
DH = 96
HID = 3072
HB = HID // P     # 24
DEPTH = 2
TQ = 1024         # own-half tokens per core
CH = 512          # token chunk
EPS = 1e-5

BF16 = ml_dtypes.bfloat16

_cache = {}


def _build():
    import concourse.bass as bass
    import concourse.mybir as mybir
    import concourse.tile as tile
    from concourse import bacc
    from concourse.masks import make_identity

    dt = mybir.dt
    nc = bacc.Bacc("TRN2", target_bir_lowering=False, debug=False, num_devices=8)

    vis = nc.dram_tensor("vis", (N_VIS + 1, D), dt.float32, kind="ExternalInput").ap()
    idx = nc.dram_tensor("idx", (T, 1), dt.int32, kind="ExternalInput").ap()
    wqkv = nc.dram_tensor("wqkv", (DEPTH, D, NH * 3 * P), dt.bfloat16, kind="ExternalInput").ap()
    bqkv = nc.dram_tensor("bqkv", (DEPTH, NH * 3, P), dt.float32, kind="ExternalInput").ap()
    wo = nc.dram_tensor("wo", (DEPTH, NH * P, D), dt.bfloat16, kind="ExternalInput").ap()
    bwo = nc.dram_tensor("bwo", (DEPTH, KD, P), dt.float32, kind="ExternalInput").ap()
    w1 = nc.dram_tensor("w1", (DEPTH, D, HID), dt.bfloat16, kind="ExternalInput").ap()
    b1 = nc.dram_tensor("b1", (DEPTH, HB, P), dt.float32, kind="ExternalInput").ap()
    w2 = nc.dram_tensor("w2", (DEPTH, HID, D), dt.bfloat16, kind="ExternalInput").ap()
    b2 = nc.dram_tensor("b2", (DEPTH, KD, P), dt.float32, kind="ExternalInput").ap()
    wdec = nc.dram_tensor("wdec", (D, D), dt.bfloat16, kind="ExternalInput").ap()
    bdec = nc.dram_tensor("bdec", (KD, P), dt.float32, kind="ExternalInput").ap()
    y = nc.dram_tensor("y", (TQ, D), dt.float32, kind="ExternalOutput").ap()

    with tile.TileContext(nc) as tc:
        from contextlib import ExitStack
        ctx = ExitStack()
        with ctx:
            const = ctx.enter_context(tc.tile_pool(name="const", bufs=1))
            xp = ctx.enter_context(tc.tile_pool(name="xp", bufs=1))
            xlnp = ctx.enter_context(tc.tile_pool(name="xlnp", bufs=1))
            qkvp = ctx.enter_context(tc.tile_pool(name="qkvp", bufs=1))
            vtp = ctx.enter_context(tc.tile_pool(name="vtp", bufs=1))
            ptp = ctx.enter_context(tc.tile_pool(name="ptp", bufs=1))
            aop = ctx.enter_context(tc.tile_pool(name="aop", bufs=1))
            hp = ctx.enter_context(tc.tile_pool(name="hp", bufs=1))
            wqp = ctx.enter_context(tc.tile_pool(name="wqp", bufs=1))
            wop = ctx.enter_context(tc.tile_pool(name="wop", bufs=1))
            w1p = ctx.enter_context(tc.tile_pool(name="w1p", bufs=2))
            w2p = ctx.enter_context(tc.tile_pool(name="w2p", bufs=3))
            tokp = ctx.enter_context(tc.tile_pool(name="tokp", bufs=2))
            scr = ctx.enter_context(tc.tile_pool(name="scr", bufs=2))
            xcp = ctx.enter_context(tc.tile_pool(name="xcp", bufs=2))
            statp = ctx.enter_context(tc.tile_pool(name="statp", bufs=1))
            dnp = ctx.enter_context(tc.tile_pool(name="dnp", bufs=2))
            idxp = ctx.enter_context(tc.tile_pool(name="idxp", bufs=2))
            biasp = ctx.enter_context(tc.tile_pool(name="biasp", bufs=1))

            psA = ctx.enter_context(tc.tile_pool(name="psA", bufs=2, space="PSUM"))
            psB = ctx.enter_context(tc.tile_pool(name="psB", bufs=1, space="PSUM"))
            psC = ctx.enter_context(tc.tile_pool(name="psC", bufs=2, space="PSUM"))
            psD = ctx.enter_context(tc.tile_pool(name="psD", bufs=3, space="PSUM"))

            f32 = dt.float32
            bf = dt.bfloat16
            f16 = dt.float16

            ones_kx1 = const.tile([P, 1], f32, tag="ones_kx1")
            nc.any.memset(ones_kx1[:], 1.0)
            ones_kx1h = const.tile([P, 1], f16, tag="ones_kx1h")
            nc.any.memset(ones_kx1h[:], 1.0)
            ones_1xp_t = const.tile([1, P], f32, tag="ones_1xp")
            nc.any.memset(ones_1xp_t[:], 1.0)
            ones_1xp = ones_1xp_t[0:1, :]
            ident_bf = const.tile([P, P], bf, tag="ident_bf")
            make_identity(nc, ident_bf[:])
            ident_f32 = const.tile([P, P], f32, tag="ident_f32")
            make_identity(nc, ident_f32[:])
            eps_t = const.tile([1, 1], f32, tag="eps")
            nc.any.memset(eps_t[:], EPS)

            # persistent activations
            x = xp.tile([P, KD, T], f16, tag="x")
            xln = xlnp.tile([P, KD, T], bf, tag="xln")
            aout = aop.tile([P, NH, T], bf, tag="aout")

            # ---- gather + transpose input: x0^T ----
            for tb in range(T // P):
                it = idxp.tile([P, 1], dt.int32, tag="idx")
                nc.sync.dma_start(it[:], idx[tb * P:(tb + 1) * P, :])
                gx = tokp.tile([P, D], f32, tag="tok")
                nc.gpsimd.indirect_dma_start(
                    out=gx[:], out_offset=None, in_=vis[:],
                    in_offset=bass.IndirectOffsetOnAxis(ap=it[:, 0:1], axis=0),
                )
                for kt in range(KD):
                    pst = psA.tile([P, P], f32, tag="a")
                    nc.tensor.transpose(pst[:], gx[:, kt * P:(kt + 1) * P], ident_f32[:])
                    nc.vector.tensor_copy(x[:, kt, tb * P:(tb + 1) * P], pst[:])

            def layer_norm(src, c0, c1, gevict):
                """LN over feature dim of src [P, KD, T] for token range [c0, c1).
                gevict(kt, cs, ce, xc_ap, r_ap) consumes normalized output."""
                # all stats at base partition 0, free-dim segments per chunk:
                # seg 0=s, 1=s2->m2, 2=mu, 3=var->std, 4=mu^2->r
                for cs in range(c0, c1, CH):
                    st = statp.tile([1, 5 * CH], f32, tag="stats")
                    sg = lambda i: st[0:1, i * CH:(i + 1) * CH]
                    ps = psC.tile([P, CH], f32, tag="c")
                    for kt in range(KD):
                        nc.tensor.matmul(ps[0:1, :], ones_kx1h[:], x[:, kt, cs:cs + CH],
                                         start=(kt == 0), stop=(kt == KD - 1))
                    nc.vector.tensor_copy(sg(0), ps[0:1, :])
                    ps2 = psC.tile([P, CH], f32, tag="c")
                    for kt in range(KD):
                        sq = scr.tile([P, CH], f32, tag="scr")
                        nc.vector.tensor_mul(sq[:], x[:, kt, cs:cs + CH], x[:, kt, cs:cs + CH])
                        nc.tensor.matmul(ps2[0:1, :], ones_kx1[:], sq[:],
                                         start=(kt == 0), stop=(kt == KD - 1))
                    nc.vector.tensor_copy(sg(1), ps2[0:1, :])
                    nc.vector.tensor_scalar_mul(sg(2), sg(0), 1.0 / D)       # mu
                    nc.vector.tensor_scalar_mul(sg(1), sg(1), 1.0 / D)       # E[x^2]
                    nc.vector.tensor_mul(sg(4), sg(2), sg(2))                # mu^2
                    nc.vector.tensor_sub(sg(3), sg(1), sg(4))                # var
                    nc.scalar.activation(sg(3), sg(3),
                                         mybir.ActivationFunctionType.Sqrt,
                                         bias=eps_t[0:1, 0:1])
                    nc.vector.reciprocal(sg(4), sg(3))                       # r
                    pmu = psC.tile([P, CH], f32, tag="c")
                    nc.tensor.matmul(pmu[:], ones_1xp, sg(2), start=True, stop=True)
                    pr = psC.tile([P, CH], f32, tag="c")
                    nc.tensor.matmul(pr[:], ones_1xp, sg(4), start=True, stop=True)
                    for kt in range(KD):
                        xc = xcp.tile([P, CH], f32, tag="xc")
                        nc.vector.tensor_sub(xc[:], x[:, kt, cs:cs + CH], pmu[:])
                        gevict(kt, cs, xc, pr)

            def ln_to_xln(kt, cs, xc, pr):
                nc.vector.tensor_mul(xln[:, kt, cs:cs + CH], xc[:], pr[:])

            for l in range(DEPTH):
                # ---------- LN1 (full range: K/V need all tokens) ----------
                layer_norm(x, 0, T, ln_to_xln)

                # qkv biases for this layer
                bq = biasp.tile([P, NH * 3], f32, tag="bq")
                nc.sync.dma_start(bq[:], bqkv[l].rearrange("a p -> p a"))
                bo_t = biasp.tile([P, KD], f32, tag="bo")
                nc.sync.dma_start(bo_t[:], bwo[l].rearrange("a p -> p a"))
                b1_t = biasp.tile([P, HB], f32, tag="b1")
                nc.sync.dma_start(b1_t[:], b1[l].rearrange("a p -> p a"))
                b2_t = biasp.tile([P, KD], f32, tag="b2")
                nc.sync.dma_start(b2_t[:], b2[l].rearrange("a p -> p a"))

                wot = wop.tile([P, NH, D], bf, tag="wo")
                for kb in range(NH):
                    nc.sync.dma_start(wot[:, kb, :], wo[l, kb * P:(kb + 1) * P, :])

                q0 = 0 if l == 0 else T - TQ    # query range start
                nq = (T - q0) // CH             # query chunks

                # ---------- attention, head-by-head ----------
                for h in range(NH):
                    wqt = wqp.tile([P, KD, 3 * P], bf, tag="wq")
                    for kt in range(KD):
                        nc.sync.dma_start(
                            wqt[:, kt, :],
                            wqkv[l, kt * P:(kt + 1) * P, h * 3 * P:(h + 1) * 3 * P])
                    qkvh = qkvp.tile([P, 3, T], bf, tag="qkvh")
                    for m in range(3):
                        m0 = q0 if m == 0 else 0
                        for cs in range(m0, T, CH):
                            ps = psA.tile([P, CH], f32, tag="a")
                            for kt in range(KD):
                                nc.tensor.matmul(ps[:], wqt[:, kt, m * P:(m + 1) * P],
                                                 xln[:, kt, cs:cs + CH],
                                                 start=(kt == 0), stop=(kt == KD - 1))
                            nc.vector.tensor_scalar_add(qkvh[:, m, cs:cs + CH], ps[:],
                                                        bq[:, h * 3 + m:h * 3 + m + 1])
                    # transpose V (and its ones-row) -> vaug [T, 128]
                    vaug = vtp.tile([P, T // P, P], bf, tag="vaug")
                    for tb in range(T // P):
                        pst = psA.tile([P, P], bf, tag="a")
                        nc.tensor.transpose(pst[:], qkvh[:, 2, tb * P:(tb + 1) * P],
                                            ident_bf[:])
                        nc.vector.tensor_copy(vaug[:, tb, :], pst[:])
                    # scores^T -> exp -> PV, per query chunk, in 2 half-passes of Tk
                    pt = ptp.tile([P, 8, CH], bf, tag="pt")
                    for cs in range(q0, T, CH):
                        pv = psB.tile([P, CH], f32, tag="b")
                        for half in range(2):
                            for tb8 in range(8):
                                tb = half * 8 + tb8
                                ps = psA.tile([P, CH], f32, tag="a")
                                nc.tensor.matmul(ps[:], qkvh[:, 1, tb * P:(tb + 1) * P],
                                                 qkvh[:, 0, cs:cs + CH],
                                                 start=True, stop=True)
                                nc.scalar.activation(pt[:, tb8, :], ps[:],
                                                     mybir.ActivationFunctionType.Exp)
                            for tb8 in range(8):
                                tb = half * 8 + tb8
                                nc.tensor.matmul(pv[:], vaug[:, tb, :], pt[:, tb8, :],
                                                 start=(tb == 0), stop=(tb == T // P - 1))
                        # normalize by denominator (row 96 of pv)
                        dn = dnp.tile([1, CH], f32, tag="dn")
                        nc.vector.tensor_copy(dn[:], pv[DH:DH + 1, :])
                        pc = psC.tile([P, CH], f32, tag="c")
                        nc.tensor.matmul(pc[:], ones_1xp, dn[:], start=True, stop=True)
                        rc = scr.tile([P, CH], f32, tag="scr")
                        nc.vector.reciprocal(rc[:], pc[:])
                        nc.vector.tensor_mul(aout[:, h, cs:cs + CH], pv[:], rc[:])

                # ---------- Wo + residual ----------
                for cs in range(q0, T, CH):
                    for m in range(KD):
                        ps = psA.tile([P, CH], f32, tag="a")
                        for kb in range(NH):
                            nc.tensor.matmul(ps[:], wot[:, kb, m * P:(m + 1) * P],
                                             aout[:, kb, cs:cs + CH],
                                             start=(kb == 0), stop=(kb == NH - 1))
                        t = scr.tile([P, CH], f32, tag="scr")
                        nc.scalar.activation(t[:], ps[:],
                                             mybir.ActivationFunctionType.Identity,
                                             bias=bo_t[:, m:m + 1])
                        nc.vector.tensor_add(x[:, m, cs:cs + CH], x[:, m, cs:cs + CH], t[:])

                # ---------- LN2 + FFN + residual ----------
                f0 = 0 if l == 0 else T - TQ
                layer_norm(x, f0, T, ln_to_xln)
                for cs in range(f0, T, CH):
                    ht = hp.tile([P, HB, CH], bf, tag="h")
                    for mg in range(8):
                        w1t = w1p.tile([P, KD, 3 * P], bf, tag="w1")
                        for kt in range(KD):
                            nc.sync.dma_start(
                                w1t[:, kt, :],
                                w1[l, kt * P:(kt + 1) * P, mg * 3 * P:(mg + 1) * 3 * P])
                        for hbl in range(3):
                            hb = mg * 3 + hbl
                            ph = psA.tile([P, CH], f32, tag="a")
                            for kt in range(KD):
                                nc.tensor.matmul(ph[:], w1t[:, kt, hbl * P:(hbl + 1) * P],
                                                 xln[:, kt, cs:cs + CH],
                                                 start=(kt == 0), stop=(kt == KD - 1))
                            nc.scalar.activation(ht[:, hb, :], ph[:],
                                                 mybir.ActivationFunctionType.Gelu,
                                                 bias=b1_t[:, hb:hb + 1])
                    for mh in range(2):
                        pds = [psD.tile([P, CH], f32, tag="d", name=f"pd{_i}") for _i in range(3)]
                        for kb in range(HB):
                            w2t = w2p.tile([P, 3 * P], bf, tag="w2")
                            nc.sync.dma_start(w2t[:],
                                              w2[l, kb * P:(kb + 1) * P,
                                                 mh * 3 * P:(mh + 1) * 3 * P])
                            for m3 in range(3):
                                nc.tensor.matmul(pds[m3][:], w2t[:, m3 * P:(m3 + 1) * P],
                                                 ht[:, kb, :],
                                                 start=(kb == 0), stop=(kb == HB - 1))
                        for m3 in range(3):
                            m = mh * 3 + m3
                            t = scr.tile([P, CH], f32, tag="scr")
                            nc.scalar.activation(t[:], pds[m3][:],
                                                 mybir.ActivationFunctionType.Identity,
                                                 bias=b2_t[:, m:m + 1])
                            nc.vector.tensor_add(x[:, m, cs:cs + CH],
                                                 x[:, m, cs:cs + CH], t[:])

            # ---------- final LN + decoder head + transpose out ----------
            layer_norm(x, T - TQ, T, ln_to_xln)
            wdt = w1p.tile([P, KD, 3 * P], bf, tag="w1")  # share slot tag with w1
            wdt2 = w1p.tile([P, KD, 3 * P], bf, tag="w1")
            for kt in range(KD):
                nc.sync.dma_start(wdt[:, kt, :], wdec[kt * P:(kt + 1) * P, 0:3 * P])
                nc.sync.dma_start(wdt2[:, kt, :], wdec[kt * P:(kt + 1) * P, 3 * P:6 * P])
            bd_t = biasp.tile([P, KD], f32, tag="bd")
            nc.sync.dma_start(bd_t[:], bdec.rearrange("a p -> p a"))
            yT = hp.tile([P, KD, CH], f32, tag="h")
            for cs in range(T - TQ, T, CH):
                for mh in range(2):
                    wsel = wdt if mh == 0 else wdt2
                    pds = [psD.tile([P, CH], f32, tag="d", name=f"pd{_i}") for _i in range(3)]
                    for m3 in range(3):
                        for kt in range(KD):
                            nc.tensor.matmul(pds[m3][:], wsel[:, kt, m3 * P:(m3 + 1) * P],
                                             xln[:, kt, cs:cs + CH],
                                             start=(kt == 0), stop=(kt == KD - 1))
                        m = mh * 3 + m3
                        nc.scalar.activation(yT[:, m, :], pds[m3][:],
                                             mybir.ActivationFunctionType.Identity,
                                             bias=bd_t[:, m:m + 1])
                for ts in range(CH // P):
                    ytok = tokp.tile([P, D], f32, tag="tok")
                    for m in range(KD):
                        pst = psA.tile([P, P], f32, tag="a")
                        nc.tensor.transpose(pst[:], yT[:, m, ts * P:(ts + 1) * P],
                                            ident_f32[:])
                        nc.vector.tensor_copy(ytok[:, m * P:(m + 1) * P], pst[:])
                    t0 = cs - (T - TQ) + ts * P
                    nc.sync.dma_start(y[t0:t0 + P, :], ytok[:])

    nc.compile()
    return nc


def _prep_weights(inputs):
    """Host-side weight folding/packing. Returns dict of shared arrays."""
    g1, be1 = inputs["gamma1"], inputs["beta1"]
    g2, be2 = inputs["gamma2"], inputs["beta2"]
    Wqkv, bqkv = inputs["Wqkv"], inputs["bqkv"]
    Wo, bo = inputs["Wo"], inputs["bo"]
    W1, b1 = inputs["W1"], inputs["b1"]
    W2, b2 = inputs["W2"], inputs["b2"]
    gn, gb = inputs["gn"], inputs["gb"]
    Wdec, bdec = inputs["Wdec"], inputs["bdec"]

    wqkv_a = np.zeros((DEPTH, D, NH * 3 * P), np.float32)
    bqkv_a = np.zeros((DEPTH, NH * 3, P), np.float32)
    wo_a = np.zeros((DEPTH, NH * P, D), np.float32)
    bwo_a = np.zeros((DEPTH, KD, P), np.float32)
    w1_a = np.zeros((DEPTH, D, HID), np.float32)
    b1_a = np.zeros((DEPTH, HB, P), np.float32)
    w2_a = np.zeros((DEPTH, HID, D), np.float32)
    b2_a = np.zeros((DEPTH, KD, P), np.float32)
    scale = 1.0 / np.sqrt(DH)
    for l in range(DEPTH):
        Wp = Wqkv[l] * g1[l][None, :]                  # fold gamma1
        bp = bqkv[l] + Wqkv[l] @ be1[l]                # fold beta1
        Wp = Wp.copy()
        bp = bp.copy()
        Wp[:D] *= scale                                # fold 1/sqrt(dh) into Q
        bp[:D] *= scale
        for h in range(NH):
            for c in range(3):                         # q,k,v
                rows = slice(c * D + h * DH, c * D + (h + 1) * DH)
                wqkv_a[l, :, (h * 3 + c) * P:(h * 3 + c) * P + DH] = Wp[rows].T
                bqkv_a[l, h * 3 + c, :DH] = bp[rows]
            bqkv_a[l, h * 3 + 2, DH] = 1.0             # ones-row -> denominators
            wo_a[l, h * P:h * P + DH, :] = Wo[l][:, h * DH:(h + 1) * DH].T
        bwo_a[l] = bo[l].reshape(KD, P)
        w1_a[l] = (W1[l] * g2[l][None, :]).T
        b1_a[l] = (b1[l] + W1[l] @ be2[l]).reshape(HB, P)
        w2_a[l] = W2[l].T
        b2_a[l] = b2[l].reshape(KD, P)
    wdec_a = (Wdec * gn[None, :]).T
    bdec_a = (bdec + Wdec @ gb).reshape(KD, P)
    return {
        "wqkv": wqkv_a.astype(BF16), "bqkv": bqkv_a,
        "wo": wo_a.astype(BF16), "bwo": bwo_a,
        "w1": w1_a.astype(BF16), "b1": b1_a,
        "w2": w2_a.astype(BF16), "b2": b2_a,
        "wdec": wdec_a.astype(BF16), "bdec": bdec_a,
    }


def kernel(**inputs):
    from concourse.bass_utils import run_bass_kernel_spmd

    inputs = {k: np.asarray(v) for k, v in inputs.items()}
    if "nc" not in _cache:
        _cache["nc"] = _build()
    nc = _cache["nc"]

    shared = _prep_weights(inputs)
    mask = inputs["mask"]
    vt = inputs["visible_tokens"].astype(np.float32)
    mt = inputs["mask_token"].astype(np.float32)

    nv = np.clip(np.cumsum(mask.astype(np.int64), axis=1) - 1, 0, N_VIS - 1)
    idx_full = np.where(mask, nv, N_VIS).astype(np.int32)     # row 512 = mask token

    in_maps = []
    for core in range(8):
        b, s = core // 2, core % 2
        if s == 0:
            perm = np.concatenate([np.arange(TQ, T), np.arange(0, TQ)])
        else:
            perm = np.arange(T)
        vis_ext = np.concatenate([vt[b], mt[None, :]], axis=0)
        m = dict(shared)
        m["vis"] = np.ascontiguousarray(vis_ext)
        m["idx"] = np.ascontiguousarray(idx_full[b][perm][:, None])
        in_maps.append(m)

    res = run_bass_kernel_spmd(nc, in_maps, core_ids=list(range(8)),
                               **_cache.get("run_kwargs", {}))
    _cache["last_results"] = res

    out = np.zeros((B, T, D), np.float32)
    for core in range(8):
        b, s = core // 2, core % 2
        out[b, s * TQ:(s + 1) * TQ] = res.results[core]["y"]
    return out


if __name__ == "__main__":
    rng = np.random.default_rng(0)
    print("building...")
    _build()
    print("built ok")


# revision 15
# speedup vs baseline: 1.3050x; 1.3050x over previous
"""Trainium2 Bass kernel for nn_DecoderHead (MAE-style decoder head).

Strategy (8 NeuronCores): data-parallel over batch B=4 x 2-way token split
per batch. Cores sharing a batch both compute layer 0 for all 2048 tokens
(cheaper than cross-core K/V exchange), then layer 1 + head for their own
1024-token half only. Tokens are permuted host-side so every core's "own"
half sits at positions 1024..2048 -- attention is permutation-equivariant,
so one NEFF serves all cores.

On-device layout is feature-major (x^T: [D, T] with D on partitions), which
makes every linear a plain lhsT.T @ rhs chain with host-pre-transposed
weights and no activation transposes. Heads are padded 96->128 so per-head
Q/K/V slices are partition-aligned; a ones-row injected into V (via the
padded bias) makes the PV matmul emit softmax denominators for free.
Softmax skips max-subtraction (|scores| <= ~2 by construction). LN
gamma/beta are folded into the adjacent weights host-side; LN stats are
computed with ones-vector matmuls on the PE and broadcast across partitions
with rank-1 matmuls.
"""

import sys
import numpy as np

sys.path.insert(0, "/opt/trn_rl_repo")

import ml_dtypes

P = 128
B = 4
N_VIS = 512
T = 2048          # N_TOT
D = 768
KD = D // P       # 6
NH = 8
DH = 96
HID = 3072
HB = HID // P     # 24
DEPTH = 2
TQ = 1024         # own-half tokens per core
CH = 512          # token chunk
EPS = 1e-5

BF16 = ml_dtypes.bfloat16

_cache = {}


def _build():
    import concourse.bass as bass
    import concourse.mybir as mybir
    import concourse.tile as tile
    from concourse import bacc
    from concourse.masks import make_identity

    dt = mybir.dt
    nc = bacc.Bacc("TRN2", target_bir_lowering=False, debug=False, num_devices=8)

    vis = nc.dram_tensor("vis", (N_VIS + 1, D), dt.float32, kind="ExternalInput").ap()
    idx = nc.dram_tensor("idx", (T, 1), dt.int32, kind="ExternalInput").ap()
    wqkv = nc.dram_tensor("wqkv", (DEPTH, D, NH * 3 * P), dt.bfloat16, kind="ExternalInput").ap()
    bqkv = nc.dram_tensor("bqkv", (DEPTH, NH * 3, P), dt.float32, kind="ExternalInput").ap()
    wo = nc.dram_tensor("wo", (DEPTH, NH * P, D), dt.bfloat16, kind="ExternalInput").ap()
    bwo = nc.dram_tensor("bwo", (DEPTH, KD, P), dt.float32, kind="ExternalInput").ap()
    w1 = nc.dram_tensor("w1", (DEPTH, D, HID), dt.bfloat16, kind="ExternalInput").ap()
    b1 = nc.dram_tensor("b1", (DEPTH, HB, P), dt.float32, kind="ExternalInput").ap()
    w2 = nc.dram_tensor("w2", (DEPTH, HID, D), dt.bfloat16, kind="ExternalInput").ap()
    b2 = nc.dram_tensor("b2", (DEPTH, KD, P), dt.float32, kind="ExternalInput").ap()
    wdec = nc.dram_tensor("wdec", (D, D), dt.bfloat16, kind="ExternalInput").ap()
    bdec = nc.dram_tensor("bdec", (KD, P), dt.float32, kind="ExternalInput").ap()
    xidx = nc.dram_tensor("xidx", (TQ, 1), dt.int32, kind="ExternalInput").ap()
    y = nc.dram_tensor("y", (TQ, D), dt.float32, kind="ExternalOutput").ap()

    with tile.TileContext(nc) as tc:
        from contextlib import ExitStack
        ctx = ExitStack()
        with ctx:
            const = ctx.enter_context(tc.tile_pool(name="const", bufs=1))
            xp = ctx.enter_context(tc.tile_pool(name="xp", bufs=1))
            xlnp = ctx.enter_context(tc.tile_pool(name="xlnp", bufs=1))
            qkvp = ctx.enter_context(tc.tile_pool(name="qkvp", bufs=1))
            vtp = ctx.enter_context(tc.tile_pool(name="vtp", bufs=1))
            ptp = ctx.enter_context(tc.tile_pool(name="ptp", bufs=1))
            aop = ctx.enter_context(tc.tile_pool(name="aop", bufs=1))
            hp = ctx.enter_context(tc.tile_pool(name="hp", bufs=1))
            wqp = ctx.enter_context(tc.tile_pool(name="wqp", bufs=1))
            wop = ctx.enter_context(tc.tile_pool(name="wop", bufs=1))
            w1p = ctx.enter_context(tc.tile_pool(name="w1p", bufs=2))
            w2p = ctx.enter_context(tc.tile_pool(name="w2p", bufs=3))
            tokp = ctx.enter_context(tc.tile_pool(name="tokp", bufs=2))
            scr = ctx.enter_context(tc.tile_pool(name="scr", bufs=2))
            xcp = ctx.enter_context(tc.tile_pool(name="xcp", bufs=2))
            statp = ctx.enter_context(tc.tile_pool(name="statp", bufs=1))
            dnp = ctx.enter_context(tc.tile_pool(name="dnp", bufs=2))
            idxp = ctx.enter_context(tc.tile_pool(name="idxp", bufs=2))
            biasp = ctx.enter_context(tc.tile_pool(name="biasp", bufs=1))

            drp = ctx.enter_context(tc.tile_pool(name="drp", bufs=1, space="DRAM"))
            psA = ctx.enter_context(tc.tile_pool(name="psA", bufs=2, space="PSUM"))
            psB = ctx.enter_context(tc.tile_pool(name="psB", bufs=1, space="PSUM"))
            psC = ctx.enter_context(tc.tile_pool(name="psC", bufs=2, space="PSUM"))
            psD = ctx.enter_context(tc.tile_pool(name="psD", bufs=3, space="PSUM"))

            f32 = dt.float32
            bf = dt.bfloat16
            f16 = dt.float16

            ones_kx1 = const.tile([P, 1], f32, tag="ones_kx1")
            nc.any.memset(ones_kx1[:], 1.0)
            ones_kx1h = const.tile([P, 1], f16, tag="ones_kx1h")
            nc.any.memset(ones_kx1h[:], 1.0)
            ones_1xp_t = const.tile([1, P], f32, tag="ones_1xp")
            nc.any.memset(ones_1xp_t[:], 1.0)
            ones_1xp = ones_1xp_t[0:1, :]
            ident_bf = const.tile([P, P], bf, tag="ident_bf")
            make_identity(nc, ident_bf[:])
            ident_f32 = const.tile([P, P], f32, tag="ident_f32")
            make_identity(nc, ident_f32[:])
            eps_t = const.tile([1, 1], f32, tag="eps")
            nc.any.memset(eps_t[:], EPS)
            ident_f16 = const.tile([P, P], f16, tag="ident_f16")
            make_identity(nc, ident_f16[:])

            # persistent activations
            x = xp.tile([P, KD, T], f16, tag="x")
            xln = xlnp.tile([P, KD, T], bf, tag="xln")
            aout = aop.tile([P, NH, TQ], bf, tag="aout")

            # ---- gather + transpose input: x0^T ----
            for tb in range(T // P):
                it = idxp.tile([P, 1], dt.int32, tag="idx")
                nc.sync.dma_start(it[:], idx[tb * P:(tb + 1) * P, :])
                gx = tokp.tile([P, D], f32, tag="tok")
                nc.gpsimd.indirect_dma_start(
                    out=gx[:], out_offset=None, in_=vis[:],
                    in_offset=bass.IndirectOffsetOnAxis(ap=it[:, 0:1], axis=0),
                )
                for kt in range(KD):
                    pst = psA.tile([P, P], f32, tag="a")
                    nc.tensor.transpose(pst[:], gx[:, kt * P:(kt + 1) * P], ident_f32[:])
                    nc.vector.tensor_copy(x[:, kt, tb * P:(tb + 1) * P], pst[:])

            def layer_norm(src, c0, c1, gevict):
                """LN over feature dim of src [P, KD, T] for token range [c0, c1).
                gevict(kt, cs, ce, xc_ap, r_ap) consumes normalized output."""
                # all stats at base partition 0, free-dim segments per chunk:
                # seg 0=s, 1=s2->m2, 2=mu, 3=var->std, 4=mu^2->r
                for cs in range(c0, c1, CH):
                    st = statp.tile([1, 5 * CH], f32, tag="stats")
                    sg = lambda i: st[0:1, i * CH:(i + 1) * CH]
                    ps = psC.tile([P, CH], f32, tag="c")
                    for kt in range(KD):
                        nc.tensor.matmul(ps[0:1, :], ones_kx1h[:], x[:, kt, cs:cs + CH],
                                         start=(kt == 0), stop=(kt == KD - 1))
                    nc.vector.tensor_copy(sg(0), ps[0:1, :])
                    ps2 = psC.tile([P, CH], f32, tag="c")
                    for kt in range(KD):
                        sq = scr.tile([P, CH], f32, tag="scr")
                        nc.vector.tensor_mul(sq[:], x[:, kt, cs:cs + CH], x[:, kt, cs:cs + CH])
                        nc.tensor.matmul(ps2[0:1, :], ones_kx1[:], sq[:],
                                         start=(kt == 0), stop=(kt == KD - 1))
                    nc.vector.tensor_copy(sg(1), ps2[0:1, :])
                    nc.vector.tensor_scalar_mul(sg(2), sg(0), 1.0 / D)       # mu
                    nc.vector.tensor_scalar_mul(sg(1), sg(1), 1.0 / D)       # E[x^2]
                    nc.vector.tensor_mul(sg(4), sg(2), sg(2))                # mu^2
                    nc.vector.tensor_sub(sg(3), sg(1), sg(4))                # var
                    nc.scalar.activation(sg(3), sg(3),
                                         mybir.ActivationFunctionType.Sqrt,
                                         bias=eps_t[0:1, 0:1])
                    nc.vector.reciprocal(sg(4), sg(3))                       # r
                    pmu = psC.tile([P, CH], f32, tag="c")
                    nc.tensor.matmul(pmu[:], ones_1xp, sg(2), start=True, stop=True)
                    pr = psC.tile([P, CH], f32, tag="c")
                    nc.tensor.matmul(pr[:], ones_1xp, sg(4), start=True, stop=True)
                    for kt in range(KD):
                        xc = xcp.tile([P, CH], f32, tag="xc")
                        nc.vector.tensor_sub(xc[:], x[:, kt, cs:cs + CH], pmu[:])
                        gevict(kt, cs, xc, pr)

            def ln_to_xln(kt, cs, xc, pr):
                nc.vector.tensor_mul(xln[:, kt, cs:cs + CH], xc[:], pr[:])

            for l in range(DEPTH):
                # ---------- LN1 (full range: K/V need all tokens) ----------
                layer_norm(x, 0, T, ln_to_xln)

                # qkv biases for this layer
                bq = biasp.tile([P, NH * 3], f32, tag="bq")
                nc.sync.dma_start(bq[:], bqkv[l].rearrange("a p -> p a"))
                bo_t = biasp.tile([P, KD], f32, tag="bo")
                nc.sync.dma_start(bo_t[:], bwo[l].rearrange("a p -> p a"))
                b1_t = biasp.tile([P, HB], f32, tag="b1")
                nc.sync.dma_start(b1_t[:], b1[l].rearrange("a p -> p a"))
                b2_t = biasp.tile([P, KD], f32, tag="b2")
                nc.sync.dma_start(b2_t[:], b2[l].rearrange("a p -> p a"))

                wot = wop.tile([P, NH, D], bf, tag="wo")
                for kb in range(NH):
                    nc.sync.dma_start(wot[:, kb, :], wo[l, kb * P:(kb + 1) * P, :])

                q0 = T - TQ                     # own-half query range


                # ---------- attention, head-by-head ----------
                for h in range(NH):
                    wqt = wqp.tile([P, KD, 3 * P], bf, tag="wq")
                    for kt in range(KD):
                        nc.sync.dma_start(
                            wqt[:, kt, :],
                            wqkv[l, kt * P:(kt + 1) * P, h * 3 * P:(h + 1) * 3 * P])
                    qkvh = qkvp.tile([P, 3, T], bf, tag="qkvh")
                    for m in range(3):
                        m0 = q0 if m == 0 else 0
                        for cs in range(m0, T, CH):
                            ps = psA.tile([P, CH], f32, tag="a")
                            for kt in range(KD):
                                nc.tensor.matmul(ps[:], wqt[:, kt, m * P:(m + 1) * P],
                                                 xln[:, kt, cs:cs + CH],
                                                 start=(kt == 0), stop=(kt == KD - 1))
                            nc.vector.tensor_scalar_add(qkvh[:, m, cs:cs + CH], ps[:],
                                                        bq[:, h * 3 + m:h * 3 + m + 1])
                    # transpose V (and its ones-row) -> vaug [T, 128]
                    vaug = vtp.tile([P, T // P, P], bf, tag="vaug")
                    for tb in range(T // P):
                        pst = psA.tile([P, P], bf, tag="a")
                        nc.tensor.transpose(pst[:], qkvh[:, 2, tb * P:(tb + 1) * P],
                                            ident_bf[:])
                        nc.vector.tensor_copy(vaug[:, tb, :], pst[:])
                    # scores^T -> exp -> PV, per query chunk, in 2 half-passes of Tk
                    pt = ptp.tile([P, 8, CH], bf, tag="pt")
                    for cs in range(q0, T, CH):
                        pv = psB.tile([P, CH], f32, tag="b")
                        for half in range(2):
                            for tb8 in range(8):
                                tb = half * 8 + tb8
                                ps = psA.tile([P, CH], f32, tag="a")
                                nc.tensor.matmul(ps[:], qkvh[:, 1, tb * P:(tb + 1) * P],
                                                 qkvh[:, 0, cs:cs + CH],
                                                 start=True, stop=True)
                                nc.scalar.activation(pt[:, tb8, :], ps[:],
                                                     mybir.ActivationFunctionType.Exp)
                            for tb8 in range(8):
                                tb = half * 8 + tb8
                                nc.tensor.matmul(pv[:], vaug[:, tb, :], pt[:, tb8, :],
                                                 start=(tb == 0), stop=(tb == T // P - 1))
                        # normalize by denominator (row 96 of pv)
                        dn = dnp.tile([1, CH], f32, tag="dn")
                        nc.vector.tensor_copy(dn[:], pv[DH:DH + 1, :])
                        pc = psC.tile([P, CH], f32, tag="c")
                        nc.tensor.matmul(pc[:], ones_1xp, dn[:], start=True, stop=True)
                        rc = scr.tile([P, CH], f32, tag="scr")
                        nc.vector.reciprocal(rc[:], pc[:])
                        nc.vector.tensor_mul(aout[:, h, cs - q0:cs - q0 + CH], pv[:], rc[:])

                # ---------- Wo + residual ----------
                for cs in range(q0, T, CH):
                    for m in range(KD):
                        ps = psA.tile([P, CH], f32, tag="a")
                        for kb in range(NH):
                            nc.tensor.matmul(ps[:], wot[:, kb, m * P:(m + 1) * P],
                                             aout[:, kb, cs - q0:cs - q0 + CH],
                                             start=(kb == 0), stop=(kb == NH - 1))
                        t = scr.tile([P, CH], f32, tag="scr")
                        nc.scalar.activation(t[:], ps[:],
                                             mybir.ActivationFunctionType.Identity,
                                             bias=bo_t[:, m:m + 1])
                        nc.vector.tensor_add(x[:, m, cs:cs + CH], x[:, m, cs:cs + CH], t[:])

                # ---------- LN2 + FFN + residual ----------
                f0 = T - TQ
                layer_norm(x, f0, T, ln_to_xln)
                for cs in range(f0, T, CH):
                    ht = hp.tile([P, HB, CH], bf, tag="h")
                    for mg in range(8):
                        w1t = w1p.tile([P, KD, 3 * P], bf, tag="w1")
                        for kt in range(KD):
                            nc.sync.dma_start(
                                w1t[:, kt, :],
                                w1[l, kt * P:(kt + 1) * P, mg * 3 * P:(mg + 1) * 3 * P])
                        for hbl in range(3):
                            hb = mg * 3 + hbl
                            ph = psA.tile([P, CH], f32, tag="a")
                            for kt in range(KD):
                                nc.tensor.matmul(ph[:], w1t[:, kt, hbl * P:(hbl + 1) * P],
                                                 xln[:, kt, cs:cs + CH],
                                                 start=(kt == 0), stop=(kt == KD - 1))
                            nc.scalar.activation(ht[:, hb, :], ph[:],
                                                 mybir.ActivationFunctionType.Gelu,
                                                 bias=b1_t[:, hb:hb + 1])
                    for mh in range(2):
                        pds = [psD.tile([P, CH], f32, tag="d", name=f"pd{_i}") for _i in range(3)]
                        for kb in range(HB):
                            w2t = w2p.tile([P, 3 * P], bf, tag="w2")
                            nc.sync.dma_start(w2t[:],
                                              w2[l, kb * P:(kb + 1) * P,
                                                 mh * 3 * P:(mh + 1) * 3 * P])
                            for m3 in range(3):
                                nc.tensor.matmul(pds[m3][:], w2t[:, m3 * P:(m3 + 1) * P],
                                                 ht[:, kb, :],
                                                 start=(kb == 0), stop=(kb == HB - 1))
                        for m3 in range(3):
                            m = mh * 3 + m3
                            t = scr.tile([P, CH], f32, tag="scr")
                            nc.scalar.activation(t[:], pds[m3][:],
                                                 mybir.ActivationFunctionType.Identity,
                                                 bias=b2_t[:, m:m + 1])
                            nc.vector.tensor_add(x[:, m, cs:cs + CH],
                                                 x[:, m, cs:cs + CH], t[:])

                if l == 0:
                    # exchange x1 halves within batch pairs
                    cc_in = drp.tile([TQ, D], f16, tag="cc_in")
                    cc_out = drp.tile([2 * TQ, D], f16, tag="cc_out")
                    for tb in range(TQ // P):
                        tk16 = tokp.tile([P, D], f16, tag="tok16")
                        for kt in range(KD):
                            pst = psA.tile([P, P], f16, tag="a")
                            nc.tensor.transpose(
                                pst[:], x[:, kt, q0 + tb * P:q0 + (tb + 1) * P],
                                ident_f16[:])
                            nc.vector.tensor_copy(tk16[:, kt * P:(kt + 1) * P], pst[:])
                        nc.sync.dma_start(cc_in[tb * P:(tb + 1) * P, :], tk16[:])
                    nc.gpsimd.collective_compute(
                        "AllGather",
                        mybir.AluOpType.bypass,
                        replica_groups=[[0, 1], [2, 3], [4, 5], [6, 7]],
                        ins=[cc_in.opt()],
                        outs=[cc_out.opt()],
                    )
                    for tb in range(TQ // P):
                        xit = idxp.tile([P, 1], dt.int32, tag="idx")
                        nc.sync.dma_start(xit[:], xidx[tb * P:(tb + 1) * P, :])
                        g16 = tokp.tile([P, D], f16, tag="tok16")
                        nc.gpsimd.indirect_dma_start(
                            out=g16[:], out_offset=None, in_=cc_out[:],
                            in_offset=bass.IndirectOffsetOnAxis(ap=xit[:, 0:1], axis=0),
                        )
                        for kt in range(KD):
                            pst = psA.tile([P, P], f16, tag="a")
                            nc.tensor.transpose(pst[:], g16[:, kt * P:(kt + 1) * P],
                                                ident_f16[:])
                            nc.vector.tensor_copy(x[:, kt, tb * P:(tb + 1) * P], pst[:])

            # ---------- final LN + decoder head + transpose out ----------
            layer_norm(x, T - TQ, T, ln_to_xln)
            wdt = w1p.tile([P, KD, 3 * P], bf, tag="w1")  # share slot tag with w1
            wdt2 = w1p.tile([P, KD, 3 * P], bf, tag="w1")
            for kt in range(KD):
                nc.sync.dma_start(wdt[:, kt, :], wdec[kt * P:(kt + 1) * P, 0:3 * P])
                nc.sync.dma_start(wdt2[:, kt, :], wdec[kt * P:(kt + 1) * P, 3 * P:6 * P])
            bd_t = biasp.tile([P, KD], f32, tag="bd")
            nc.sync.dma_start(bd_t[:], bdec.rearrange("a p -> p a"))
            yT = hp.tile([P, KD, CH], f32, tag="h")
            for cs in range(T - TQ, T, CH):
                for mh in range(2):
                    wsel = wdt if mh == 0 else wdt2
                    pds = [psD.tile([P, CH], f32, tag="d", name=f"pd{_i}") for _i in range(3)]
                    for m3 in range(3):
                        for kt in range(KD):
                            nc.tensor.matmul(pds[m3][:], wsel[:, kt, m3 * P:(m3 + 1) * P],
                                             xln[:, kt, cs:cs + CH],
                                             start=(kt == 0), stop=(kt == KD - 1))
                        m = mh * 3 + m3
                        nc.scalar.activation(yT[:, m, :], pds[m3][:],
                                             mybir.ActivationFunctionType.Identity,
                                             bias=bd_t[:, m:m + 1])
                for ts in range(CH // P):
                    ytok = tokp.tile([P, D], f32, tag="tok")
                    for m in range(KD):
                        pst = psA.tile([P, P], f32, tag="a")
                        nc.tensor.transpose(pst[:], yT[:, m, ts * P:(ts + 1) * P],
                                            ident_f32[:])
                        nc.vector.tensor_copy(ytok[:, m * P:(m + 1) * P], pst[:])
                    t0 = cs - (T - TQ) + ts * P
                    nc.sync.dma_start(y[t0:t0 + P, :], ytok[:])

    nc.compile()
    return nc


def _prep_weights(inputs):
    """Host-side weight folding/packing. Returns dict of shared arrays."""
    g1, be1 = inputs["gamma1"], inputs["beta1"]
    g2, be2 = inputs["gamma2"], inputs["beta2"]
    Wqkv, bqkv = inputs["Wqkv"], inputs["bqkv"]
    Wo, bo = inputs["Wo"], inputs["bo"]
    W1, b1 = inputs["W1"], inputs["b1"]
    W2, b2 = inputs["W2"], inputs["b2"]
    gn, gb = inputs["gn"], inputs["gb"]
    Wdec, bdec = inputs["Wdec"], inputs["bdec"]

    wqkv_a = np.zeros((DEPTH, D, NH * 3 * P), np.float32)
    bqkv_a = np.zeros((DEPTH, NH * 3, P), np.float32)
    wo_a = np.zeros((DEPTH, NH * P, D), np.float32)
    bwo_a = np.zeros((DEPTH, KD, P), np.float32)
    w1_a = np.zeros((DEPTH, D, HID), np.float32)
    b1_a = np.zeros((DEPTH, HB, P), np.float32)
    w2_a = np.zeros((DEPTH, HID, D), np.float32)
    b2_a = np.zeros((DEPTH, KD, P), np.float32)
    scale = 1.0 / np.sqrt(DH)
    for l in range(DEPTH):
        Wp = Wqkv[l] * g1[l][None, :]                  # fold gamma1
        bp = bqkv[l] + Wqkv[l] @ be1[l]                # fold beta1
        Wp = Wp.copy()
        bp = bp.copy()
        Wp[:D] *= scale                                # fold 1/sqrt(dh) into Q
        bp[:D] *= scale
        for h in range(NH):
            for c in range(3):                         # q,k,v
                rows = slice(c * D + h * DH, c * D + (h + 1) * DH)
                wqkv_a[l, :, (h * 3 + c) * P:(h * 3 + c) * P + DH] = Wp[rows].T
                bqkv_a[l, h * 3 + c, :DH] = bp[rows]
            bqkv_a[l, h * 3 + 2, DH] = 1.0             # ones-row -> denominators
            wo_a[l, h * P:h * P + DH, :] = Wo[l][:, h * DH:(h + 1) * DH].T
        bwo_a[l] = bo[l].reshape(KD, P)
        w1_a[l] = (W1[l] * g2[l][None, :]).T
        b1_a[l] = (b1[l] + W1[l] @ be2[l]).reshape(HB, P)
        w2_a[l] = W2[l].T
        b2_a[l] = b2[l].reshape(KD, P)
    wdec_a = (Wdec * gn[None, :]).T
    bdec_a = (bdec + Wdec @ gb).reshape(KD, P)
    return {
        "wqkv": wqkv_a.astype(BF16), "bqkv": bqkv_a,
        "wo": wo_a.astype(BF16), "bwo": bwo_a,
        "w1": w1_a.astype(BF16), "b1": b1_a,
        "w2": w2_a.astype(BF16), "b2": b2_a,
        "wdec": wdec_a.astype(BF16), "bdec": bdec_a,
    }


def kernel(**inputs):
    from concourse.bass_utils import run_bass_kernel_spmd

    inputs = {k: np.asarray(v) for k, v in inputs.items()}
    if "nc" not in _cache:
        _cache["nc"] = _build()
    nc = _cache["nc"]

    shared = _prep_weights(inputs)
    mask = inputs["mask"]
    vt = inputs["visible_tokens"].astype(np.float32)
    mt = inputs["mask_token"].astype(np.float32)

    nv = np.clip(np.cumsum(mask.astype(np.int64), axis=1) - 1, 0, N_VIS - 1)
    idx_full = np.where(mask, nv, N_VIS).astype(np.int32)     # row 512 = mask token

    in_maps = []
    for core in range(8):
        b, s = core // 2, core % 2
        if s == 0:
            perm = np.concatenate([np.arange(TQ, T), np.arange(0, TQ)])
        else:
            perm = np.arange(T)
        vis_ext = np.concatenate([vt[b], mt[None, :]], axis=0)
        m = dict(shared)
        m["vis"] = np.ascontiguousarray(vis_ext)
        m["idx"] = np.ascontiguousarray(idx_full[b][perm][:, None])
        m["xidx"] = np.ascontiguousarray(perm[:TQ].astype(np.int32)[:, None])
        in_maps.append(m)

    res = run_bass_kernel_spmd(nc, in_maps, core_ids=list(range(8)),
                               **_cache.get("run_kwargs", {}))
    _cache["last_results"] = res

    out = np.zeros((B, T, D), np.float32)
    for core in range(8):
        b, s = core // 2, core % 2
        out[b, s * TQ:(s + 1) * TQ] = res.results[core]["y"]
    return out


if __name__ == "__main__":
    rng = np.random.default_rng(0)
    print("building...")
    _build()
    print("built ok")


# revision 16
# speedup vs baseline: 1.4789x; 1.1333x over previous
"""Trainium2 Bass kernel for nn_DecoderHead (MAE-style decoder head).

Strategy (8 NeuronCores): data-parallel over batch B=4 x 2-way token split
per batch. Cores sharing a batch both compute layer 0 for all 2048 tokens
(cheaper than cross-core K/V exchange), then layer 1 + head for their own
1024-token half only. Tokens are permuted host-side so every core's "own"
half sits at positions 1024..2048 -- attention is permutation-equivariant,
so one NEFF serves all cores.

On-device layout is feature-major (x^T: [D, T] with D on partitions), which
makes every linear a plain lhsT.T @ rhs chain with host-pre-transposed
weights and no activation transposes. Heads are padded 96->128 so per-head
Q/K/V slices are partition-aligned; a ones-row injected into V (via the
padded bias) makes the PV matmul emit softmax denominators for free.
Softmax skips max-subtraction (|scores| <= ~2 by construction). LN
gamma/beta are folded into the adjacent weights host-side; LN stats are
computed with ones-vector matmuls on the PE and broadcast across partitions
with rank-1 matmuls.
"""

import sys
import numpy as np

sys.path.insert(0, "/opt/trn_rl_repo")

import ml_dtypes

P = 128
B = 4
N_VIS = 512
T = 2048          # N_TOT
D = 768
KD = D // P       # 6
NH = 8
DH = 96
HID = 3072
HB = HID // P     # 24
DEPTH = 2
TQ = 1024         # own-half tokens per core
CH = 512          # token chunk
EPS = 1e-5

BF16 = ml_dtypes.bfloat16

_cache = {}


def _build():
    import concourse.bass as bass
    import concourse.mybir as mybir
    import concourse.tile as tile
    from concourse import bacc
    from concourse.masks import make_identity

    dt = mybir.dt
    nc = bacc.Bacc("TRN2", target_bir_lowering=False, debug=False, num_devices=8)

    vis = nc.dram_tensor("vis", (N_VIS + 1, D), dt.float32, kind="ExternalInput").ap()
    idx = nc.dram_tensor("idx", (T, 1), dt.int32, kind="ExternalInput").ap()
    wqkv = nc.dram_tensor("wqkv", (DEPTH, D, NH * 3 * P), dt.bfloat16, kind="ExternalInput").ap()
    bqkv = nc.dram_tensor("bqkv", (DEPTH, NH * 3, P), dt.float32, kind="ExternalInput").ap()
    wo = nc.dram_tensor("wo", (DEPTH, NH * P, D), dt.bfloat16, kind="ExternalInput").ap()
    bwo = nc.dram_tensor("bwo", (DEPTH, KD, P), dt.float32, kind="ExternalInput").ap()
    w1 = nc.dram_tensor("w1", (DEPTH, D, HID), dt.bfloat16, kind="ExternalInput").ap()
    b1 = nc.dram_tensor("b1", (DEPTH, HB, P), dt.float32, kind="ExternalInput").ap()
    w2 = nc.dram_tensor("w2", (DEPTH, HID, D), dt.bfloat16, kind="ExternalInput").ap()
    b2 = nc.dram_tensor("b2", (DEPTH, KD, P), dt.float32, kind="ExternalInput").ap()
    wdec = nc.dram_tensor("wdec", (D, D), dt.bfloat16, kind="ExternalInput").ap()
    bdec = nc.dram_tensor("bdec", (KD, P), dt.float32, kind="ExternalInput").ap()
    xidx = nc.dram_tensor("xidx", (TQ, 1), dt.int32, kind="ExternalInput").ap()
    y = nc.dram_tensor("y", (TQ, D), dt.float32, kind="ExternalOutput").ap()

    with tile.TileContext(nc) as tc:
        from contextlib import ExitStack
        ctx = ExitStack()
        with ctx:
            const = ctx.enter_context(tc.tile_pool(name="const", bufs=1))
            xp = ctx.enter_context(tc.tile_pool(name="xp", bufs=1))
            xlnp = ctx.enter_context(tc.tile_pool(name="xlnp", bufs=1))
            qkvp = ctx.enter_context(tc.tile_pool(name="qkvp", bufs=2))
            vtp = ctx.enter_context(tc.tile_pool(name="vtp", bufs=1))
            ptp = ctx.enter_context(tc.tile_pool(name="ptp", bufs=1))
            aop = ctx.enter_context(tc.tile_pool(name="aop", bufs=1))
            hp = ctx.enter_context(tc.tile_pool(name="hp", bufs=1))
            wqp = ctx.enter_context(tc.tile_pool(name="wqp", bufs=2))
            wop = ctx.enter_context(tc.tile_pool(name="wop", bufs=1))
            w1p = ctx.enter_context(tc.tile_pool(name="w1p", bufs=2))
            w2p = ctx.enter_context(tc.tile_pool(name="w2p", bufs=3))
            tokp = ctx.enter_context(tc.tile_pool(name="tokp", bufs=2))
            scr = ctx.enter_context(tc.tile_pool(name="scr", bufs=2))
            xcp = ctx.enter_context(tc.tile_pool(name="xcp", bufs=2))
            statp = ctx.enter_context(tc.tile_pool(name="statp", bufs=1))
            dnp = ctx.enter_context(tc.tile_pool(name="dnp", bufs=2))
            idxp = ctx.enter_context(tc.tile_pool(name="idxp", bufs=2))
            biasp = ctx.enter_context(tc.tile_pool(name="biasp", bufs=1))

            drp = ctx.enter_context(tc.tile_pool(name="drp", bufs=1, space="DRAM"))
            psA = ctx.enter_context(tc.tile_pool(name="psA", bufs=3, space="PSUM"))
            psC = ctx.enter_context(tc.tile_pool(name="psC", bufs=2, space="PSUM"))
            psD = ctx.enter_context(tc.tile_pool(name="psD", bufs=3, space="PSUM"))

            f32 = dt.float32
            bf = dt.bfloat16
            f16 = dt.float16

            ones_kx1 = const.tile([P, 1], f32, tag="ones_kx1")
            nc.any.memset(ones_kx1[:], 1.0)
            ones_kx1h = const.tile([P, 1], f16, tag="ones_kx1h")
            nc.any.memset(ones_kx1h[:], 1.0)
            ones_1xp_t = const.tile([1, P], f32, tag="ones_1xp")
            nc.any.memset(ones_1xp_t[:], 1.0)
            ones_1xp = ones_1xp_t[0:1, :]
            ident_bf = const.tile([P, P], bf, tag="ident_bf")
            make_identity(nc, ident_bf[:])
            ident_f32 = const.tile([P, P], f32, tag="ident_f32")
            make_identity(nc, ident_f32[:])
            eps_t = const.tile([1, 1], f32, tag="eps")
            nc.any.memset(eps_t[:], EPS)
            ident_f16 = const.tile([P, P], f16, tag="ident_f16")
            make_identity(nc, ident_f16[:])

            # persistent activations
            x = xp.tile([P, KD, T], f16, tag="x")
            xln = xlnp.tile([P, KD, T], bf, tag="xln")
            aout = aop.tile([P, NH, TQ], bf, tag="aout")

            # ---- gather + transpose input: x0^T ----
            for tb in range(T // P):
                it = idxp.tile([P, 1], dt.int32, tag="idx")
                nc.sync.dma_start(it[:], idx[tb * P:(tb + 1) * P, :])
                gx = tokp.tile([P, D], f32, tag="tok")
                nc.gpsimd.indirect_dma_start(
                    out=gx[:], out_offset=None, in_=vis[:],
                    in_offset=bass.IndirectOffsetOnAxis(ap=it[:, 0:1], axis=0),
                )
                for kt in range(KD):
                    pst = psA.tile([P, P], f32, tag="a")
                    nc.tensor.transpose(pst[:], gx[:, kt * P:(kt + 1) * P], ident_f32[:])
                    nc.vector.tensor_copy(x[:, kt, tb * P:(tb + 1) * P], pst[:])

            def layer_norm(src, c0, c1, gevict):
                """LN over feature dim of src [P, KD, T] for token range [c0, c1).
                gevict(kt, cs, ce, xc_ap, r_ap) consumes normalized output."""
                # all stats at base partition 0, free-dim segments per chunk:
                # seg 0=s, 1=s2->m2, 2=mu, 3=var->std, 4=mu^2->r
                for cs in range(c0, c1, CH):
                    st = statp.tile([1, 5 * CH], f32, tag="stats")
                    sg = lambda i: st[0:1, i * CH:(i + 1) * CH]
                    ps = psC.tile([P, CH], f32, tag="c")
                    for kt in range(KD):
                        nc.tensor.matmul(ps[0:1, :], ones_kx1h[:], x[:, kt, cs:cs + CH],
                                         start=(kt == 0), stop=(kt == KD - 1))
                    nc.vector.tensor_copy(sg(0), ps[0:1, :])
                    ps2 = psC.tile([P, CH], f32, tag="c")
                    for kt in range(KD):
                        sq = scr.tile([P, CH], f32, tag="scr")
                        nc.vector.tensor_mul(sq[:], x[:, kt, cs:cs + CH], x[:, kt, cs:cs + CH])
                        nc.tensor.matmul(ps2[0:1, :], ones_kx1[:], sq[:],
                                         start=(kt == 0), stop=(kt == KD - 1))
                    nc.vector.tensor_copy(sg(1), ps2[0:1, :])
                    nc.vector.tensor_scalar_mul(sg(2), sg(0), 1.0 / D)       # mu
                    nc.vector.tensor_scalar_mul(sg(1), sg(1), 1.0 / D)       # E[x^2]
                    nc.vector.tensor_mul(sg(4), sg(2), sg(2))                # mu^2
                    nc.vector.tensor_sub(sg(3), sg(1), sg(4))                # var
                    nc.scalar.activation(sg(3), sg(3),
                                         mybir.ActivationFunctionType.Sqrt,
                                         bias=eps_t[0:1, 0:1])
                    nc.vector.reciprocal(sg(4), sg(3))                       # r
                    pmu = psC.tile([P, CH], f32, tag="c")
                    nc.tensor.matmul(pmu[:], ones_1xp, sg(2), start=True, stop=True)
                    pr = psC.tile([P, CH], f32, tag="c")
                    nc.tensor.matmul(pr[:], ones_1xp, sg(4), start=True, stop=True)
                    for kt in range(KD):
                        xc = xcp.tile([P, CH], f32, tag="xc")
                        nc.vector.tensor_sub(xc[:], x[:, kt, cs:cs + CH], pmu[:])
                        gevict(kt, cs, xc, pr)

            def ln_to_xln(kt, cs, xc, pr):
                nc.vector.tensor_mul(xln[:, kt, cs:cs + CH], xc[:], pr[:])

            for l in range(DEPTH):
                # ---------- LN1 (full range: K/V need all tokens) ----------
                layer_norm(x, 0, T, ln_to_xln)

                # qkv biases for this layer
                bq = biasp.tile([P, NH * 3], f32, tag="bq")
                nc.sync.dma_start(bq[:], bqkv[l].rearrange("a p -> p a"))
                bo_t = biasp.tile([P, KD], f32, tag="bo")
                nc.sync.dma_start(bo_t[:], bwo[l].rearrange("a p -> p a"))
                b1_t = biasp.tile([P, HB], f32, tag="b1")
                nc.sync.dma_start(b1_t[:], b1[l].rearrange("a p -> p a"))
                b2_t = biasp.tile([P, KD], f32, tag="b2")
                nc.sync.dma_start(b2_t[:], b2[l].rearrange("a p -> p a"))

                wot = wop.tile([P, NH, D], bf, tag="wo")
                for kb in range(NH):
                    nc.sync.dma_start(wot[:, kb, :], wo[l, kb * P:(kb + 1) * P, :])

                q0 = T - TQ                     # own-half query range


                # ---------- attention, head-by-head ----------
                for h in range(NH):
                    wqt = wqp.tile([P, KD, 3 * P], bf, tag="wq")
                    for kt in range(KD):
                        nc.sync.dma_start(
                            wqt[:, kt, :],
                            wqkv[l, kt * P:(kt + 1) * P, h * 3 * P:(h + 1) * 3 * P])
                    qkvh = qkvp.tile([P, 3, T], bf, tag="qkvh")
                    for m in range(3):
                        m0 = q0 if m == 0 else 0
                        for cs in range(m0, T, CH):
                            ps = psA.tile([P, CH], f32, tag="a")
                            for kt in range(KD):
                                nc.tensor.matmul(ps[:], wqt[:, kt, m * P:(m + 1) * P],
                                                 xln[:, kt, cs:cs + CH],
                                                 start=(kt == 0), stop=(kt == KD - 1))
                            nc.vector.tensor_scalar_add(qkvh[:, m, cs:cs + CH], ps[:],
                                                        bq[:, h * 3 + m:h * 3 + m + 1])
                    # transpose V (and its ones-row) -> vaug [T, 128]
                    vaug = vtp.tile([P, T // P, P], bf, tag="vaug")
                    for tb in range(T // P):
                        pst = psA.tile([P, P], bf, tag="a")
                        nc.tensor.transpose(pst[:], qkvh[:, 2, tb * P:(tb + 1) * P],
                                            ident_bf[:])
                        nc.vector.tensor_copy(vaug[:, tb, :], pst[:])
                    # scores^T -> exp -> PV, per query chunk, in 2 half-passes of Tk
                    pt = ptp.tile([P, 8, CH], bf, tag="pt")
                    for cs in range(q0, T, CH):
                        pv = psD.tile([P, CH], f32, tag="d")
                        for half in range(2):
                            for tb8 in range(8):
                                tb = half * 8 + tb8
                                ps = psA.tile([P, CH], f32, tag="a")
                                nc.tensor.matmul(ps[:], qkvh[:, 1, tb * P:(tb + 1) * P],
                                                 qkvh[:, 0, cs:cs + CH],
                                                 start=True, stop=True)
                                nc.scalar.activation(pt[:, tb8, :], ps[:],
                                                     mybir.ActivationFunctionType.Exp)
                            for tb8 in range(8):
                                tb = half * 8 + tb8
                                nc.tensor.matmul(pv[:], vaug[:, tb, :], pt[:, tb8, :],
                                                 start=(tb == 0), stop=(tb == T // P - 1))
                        # normalize by denominator (row 96 of pv)
                        dn = dnp.tile([1, CH], f32, tag="dn")
                        nc.vector.tensor_copy(dn[:], pv[DH:DH + 1, :])
                        pc = psC.tile([P, CH], f32, tag="c")
                        nc.tensor.matmul(pc[:], ones_1xp, dn[:], start=True, stop=True)
                        rc = scr.tile([P, CH], f32, tag="scr")
                        nc.vector.reciprocal(rc[:], pc[:])
                        nc.vector.tensor_mul(aout[:, h, cs - q0:cs - q0 + CH], pv[:], rc[:])

                # ---------- Wo + residual ----------
                for cs in range(q0, T, CH):
                    for m in range(KD):
                        ps = psA.tile([P, CH], f32, tag="a")
                        for kb in range(NH):
                            nc.tensor.matmul(ps[:], wot[:, kb, m * P:(m + 1) * P],
                                             aout[:, kb, cs - q0:cs - q0 + CH],
                                             start=(kb == 0), stop=(kb == NH - 1))
                        t = scr.tile([P, CH], f32, tag="scr")
                        nc.scalar.activation(t[:], ps[:],
                                             mybir.ActivationFunctionType.Identity,
                                             bias=bo_t[:, m:m + 1])
                        nc.vector.tensor_add(x[:, m, cs:cs + CH], x[:, m, cs:cs + CH], t[:])

                # ---------- LN2 + FFN + residual ----------
                f0 = T - TQ
                layer_norm(x, f0, T, ln_to_xln)
                for cs in range(f0, T, CH):
                    ht = hp.tile([P, HB, CH], bf, tag="h")
                    for mg in range(8):
                        w1t = w1p.tile([P, KD, 3 * P], bf, tag="w1")
                        for kt in range(KD):
                            nc.sync.dma_start(
                                w1t[:, kt, :],
                                w1[l, kt * P:(kt + 1) * P, mg * 3 * P:(mg + 1) * 3 * P])
                        for hbl in range(3):
                            hb = mg * 3 + hbl
                            ph = psA.tile([P, CH], f32, tag="a")
                            for kt in range(KD):
                                nc.tensor.matmul(ph[:], w1t[:, kt, hbl * P:(hbl + 1) * P],
                                                 xln[:, kt, cs:cs + CH],
                                                 start=(kt == 0), stop=(kt == KD - 1))
                            nc.scalar.activation(ht[:, hb, :], ph[:],
                                                 mybir.ActivationFunctionType.Gelu,
                                                 bias=b1_t[:, hb:hb + 1])
                    for mh in range(2):
                        pds = [psD.tile([P, CH], f32, tag="d", name=f"pd{_i}") for _i in range(3)]
                        for kb in range(HB):
                            w2t = w2p.tile([P, 3 * P], bf, tag="w2")
                            nc.sync.dma_start(w2t[:],
                                              w2[l, kb * P:(kb + 1) * P,
                                                 mh * 3 * P:(mh + 1) * 3 * P])
                            for m3 in range(3):
                                nc.tensor.matmul(pds[m3][:], w2t[:, m3 * P:(m3 + 1) * P],
                                                 ht[:, kb, :],
                                                 start=(kb == 0), stop=(kb == HB - 1))
                        for m3 in range(3):
                            m = mh * 3 + m3
                            t = scr.tile([P, CH], f32, tag="scr")
                            nc.scalar.activation(t[:], pds[m3][:],
                                                 mybir.ActivationFunctionType.Identity,
                                                 bias=b2_t[:, m:m + 1])
                            nc.vector.tensor_add(x[:, m, cs:cs + CH],
                                                 x[:, m, cs:cs + CH], t[:])

                if l == 0:
                    # exchange x1 halves within batch pairs
                    cc_in = drp.tile([TQ, D], f16, tag="cc_in")
                    cc_out = drp.tile([2 * TQ, D], f16, tag="cc_out")
                    for tb in range(TQ // P):
                        tk16 = tokp.tile([P, D], f16, tag="tok16")
                        for kt in range(KD):
                            pst = psA.tile([P, P], f16, tag="a")
                            nc.tensor.transpose(
                                pst[:], x[:, kt, q0 + tb * P:q0 + (tb + 1) * P],
                                ident_f16[:])
                            nc.vector.tensor_copy(tk16[:, kt * P:(kt + 1) * P], pst[:])
                        nc.sync.dma_start(cc_in[tb * P:(tb + 1) * P, :], tk16[:])
                    nc.gpsimd.collective_compute(
                        "AllGather",
                        mybir.AluOpType.bypass,
                        replica_groups=[[0, 1], [2, 3], [4, 5], [6, 7]],
                        ins=[cc_in.opt()],
                        outs=[cc_out.opt()],
                    )
                    for tb in range(TQ // P):
                        xit = idxp.tile([P, 1], dt.int32, tag="idx")
                        nc.sync.dma_start(xit[:], xidx[tb * P:(tb + 1) * P, :])
                        g16 = tokp.tile([P, D], f16, tag="tok16")
                        nc.gpsimd.indirect_dma_start(
                            out=g16[:], out_offset=None, in_=cc_out[:],
                            in_offset=bass.IndirectOffsetOnAxis(ap=xit[:, 0:1], axis=0),
                        )
                        for kt in range(KD):
                            pst = psA.tile([P, P], f16, tag="a")
                            nc.tensor.transpose(pst[:], g16[:, kt * P:(kt + 1) * P],
                                                ident_f16[:])
                            nc.vector.tensor_copy(x[:, kt, tb * P:(tb + 1) * P], pst[:])

            # ---------- final LN + decoder head + transpose out ----------
            layer_norm(x, T - TQ, T, ln_to_xln)
            wdt = w1p.tile([P, KD, 3 * P], bf, tag="w1")  # share slot tag with w1
            wdt2 = w1p.tile([P, KD, 3 * P], bf, tag="w1")
            for kt in range(KD):
                nc.sync.dma_start(wdt[:, kt, :], wdec[kt * P:(kt + 1) * P, 0:3 * P])
                nc.sync.dma_start(wdt2[:, kt, :], wdec[kt * P:(kt + 1) * P, 3 * P:6 * P])
            bd_t = biasp.tile([P, KD], f32, tag="bd")
            nc.sync.dma_start(bd_t[:], bdec.rearrange("a p -> p a"))
            yT = hp.tile([P, KD, CH], f32, tag="h")
            for cs in range(T - TQ, T, CH):
                for mh in range(2):
                    wsel = wdt if mh == 0 else wdt2
                    pds = [psD.tile([P, CH], f32, tag="d", name=f"pd{_i}") for _i in range(3)]
                    for m3 in range(3):
                        for kt in range(KD):
                            nc.tensor.matmul(pds[m3][:], wsel[:, kt, m3 * P:(m3 + 1) * P],
                                             xln[:, kt, cs:cs + CH],
                                             start=(kt == 0), stop=(kt == KD - 1))
                        m = mh * 3 + m3
                        nc.scalar.activation(yT[:, m, :], pds[m3][:],
                                             mybir.ActivationFunctionType.Identity,
                                             bias=bd_t[:, m:m + 1])
                for ts in range(CH // P):
                    ytok = tokp.tile([P, D], f32, tag="tok")
                    for m in range(KD):
                        pst = psA.tile([P, P], f32, tag="a")
                        nc.tensor.transpose(pst[:], yT[:, m, ts * P:(ts + 1) * P],
                                            ident_f32[:])
                        nc.vector.tensor_copy(ytok[:, m * P:(m + 1) * P], pst[:])
                    t0 = cs - (T - TQ) + ts * P
                    nc.sync.dma_start(y[t0:t0 + P, :], ytok[:])

    nc.compile()
    return nc


def _prep_weights(inputs):
    """Host-side weight folding/packing. Returns dict of shared arrays."""
    g1, be1 = inputs["gamma1"], inputs["beta1"]
    g2, be2 = inputs["gamma2"], inputs["beta2"]
    Wqkv, bqkv = inputs["Wqkv"], inputs["bqkv"]
    Wo, bo = inputs["Wo"], inputs["bo"]
    W1, b1 = inputs["W1"], inputs["b1"]
    W2, b2 = inputs["W2"], inputs["b2"]
    gn, gb = inputs["gn"], inputs["gb"]
    Wdec, bdec = inputs["Wdec"], inputs["bdec"]

    wqkv_a = np.zeros((DEPTH, D, NH * 3 * P), np.float32)
    bqkv_a = np.zeros((DEPTH, NH * 3, P), np.float32)
    wo_a = np.zeros((DEPTH, NH * P, D), np.float32)
    bwo_a = np.zeros((DEPTH, KD, P), np.float32)
    w1_a = np.zeros((DEPTH, D, HID), np.float32)
    b1_a = np.zeros((DEPTH, HB, P), np.float32)
    w2_a = np.zeros((DEPTH, HID, D), np.float32)
    b2_a = np.zeros((DEPTH, KD, P), np.float32)
    scale = 1.0 / np.sqrt(DH)
    for l in range(DEPTH):
        Wp = Wqkv[l] * g1[l][None, :]                  # fold gamma1
        bp = bqkv[l] + Wqkv[l] @ be1[l]                # fold beta1
        Wp = Wp.copy()
        bp = bp.copy()
        Wp[:D] *= scale                                # fold 1/sqrt(dh) into Q
        bp[:D] *= scale
        for h in range(NH):
            for c in range(3):                         # q,k,v
                rows = slice(c * D + h * DH, c * D + (h + 1) * DH)
                wqkv_a[l, :, (h * 3 + c) * P:(h * 3 + c) * P + DH] = Wp[rows].T
                bqkv_a[l, h * 3 + c, :DH] = bp[rows]
            bqkv_a[l, h * 3 + 2, DH] = 1.0             # ones-row -> denominators
            wo_a[l, h * P:h * P + DH, :] = Wo[l][:, h * DH:(h + 1) * DH].T
        bwo_a[l] = bo[l].reshape(KD, P)
        w1_a[l] = (W1[l] * g2[l][None, :]).T
        b1_a[l] = (b1[l] + W1[l] @ be2[l]).reshape(HB, P)
        w2_a[l] = W2[l].T
        b2_a[l] = b2[l].reshape(KD, P)
    wdec_a = (Wdec * gn[None, :]).T
    bdec_a = (bdec + Wdec @ gb).reshape(KD, P)
    return {
        "wqkv": wqkv_a.astype(BF16), "bqkv": bqkv_a,
        "wo": wo_a.astype(BF16), "bwo": bwo_a,
        "w1": w1_a.astype(BF16), "b1": b1_a,
        "w2": w2_a.astype(BF16), "b2": b2_a,
        "wdec": wdec_a.astype(BF16), "bdec": bdec_a,
    }


def kernel(**inputs):
    from concourse.bass_utils import run_bass_kernel_spmd

    inputs = {k: np.asarray(v) for k, v in inputs.items()}
    if "nc" not in _cache:
        _cache["nc"] = _build()
    nc = _cache["nc"]

    shared = _prep_weights(inputs)
    mask = inputs["mask"]
    vt = inputs["visible_tokens"].astype(np.float32)
    mt = inputs["mask_token"].astype(np.float32)

    nv = np.clip(np.cumsum(mask.astype(np.int64), axis=1) - 1, 0, N_VIS - 1)
    idx_full = np.where(mask, nv, N_VIS).astype(np.int32)     # row 512 = mask token

    in_maps = []
    for core in range(8):
        b, s = core // 2, core % 2
        if s == 0:
            perm = np.concatenate([np.arange(TQ, T), np.arange(0, TQ)])
        else:
            perm = np.arange(T)
        vis_ext = np.concatenate([vt[b], mt[None, :]], axis=0)
        m = dict(shared)
        m["vis"] = np.ascontiguousarray(vis_ext)
        m["idx"] = np.ascontiguousarray(idx_full[b][perm][:, None])
        m["xidx"] = np.ascontiguousarray(perm[:TQ].astype(np.int32)[:, None])
        in_maps.append(m)

    res = run_bass_kernel_spmd(nc, in_maps, core_ids=list(range(8)),
                               **_cache.get("run_kwargs", {}))
    _cache["last_results"] = res

    out = np.zeros((B, T, D), np.float32)
    for core in range(8):
        b, s = core // 2, core % 2
        out[b, s * TQ:(s + 1) * TQ] = res.results[core]["y"]
    return out


if __name__ == "__main__":
    rng = np.random.default_rng(0)
    print("building...")
    _build()
    print("built ok")


# revision 17
# speedup vs baseline: 1.4810x; 1.0014x over previous
"""Trainium2 Bass kernel for nn_DecoderHead (MAE-style decoder head).

Strategy (8 NeuronCores): data-parallel over batch B=4 x 2-way token split
per batch. Cores sharing a batch both compute layer 0 for all 2048 tokens
(cheaper than cross-core K/V exchange), then layer 1 + head for their own
1024-token half only. Tokens are permuted host-side so every core's "own"
half sits at positions 1024..2048 -- attention is permutation-equivariant,
so one NEFF serves all cores.

On-device layout is feature-major (x^T: [D, T] with D on partitions), which
makes every linear a plain lhsT.T @ rhs chain with host-pre-transposed
weights and no activation transposes. Heads are padded 96->128 so per-head
Q/K/V slices are partition-aligned; a ones-row injected into V (via the
padded bias) makes the PV matmul emit softmax denominators for free.
Softmax skips max-subtraction (|scores| <= ~2 by construction). LN
gamma/beta are folded into the adjacent weights host-side; LN stats are
computed with ones-vector matmuls on the PE and broadcast across partitions
with rank-1 matmuls.
"""

import sys
import numpy as np

sys.path.insert(0, "/opt/trn_rl_repo")

import ml_dtypes

P = 128
B = 4
N_VIS = 512
T = 2048          # N_TOT
D = 768
KD = D // P       # 6
NH = 8
DH = 96
HID = 3072
HB = HID // P     # 24
DEPTH = 2
TQ = 1024         # own-half tokens per core
CH = 512          # token chunk
EPS = 1e-5

BF16 = ml_dtypes.bfloat16

_cache = {}


def _build():
    import concourse.bass as bass
    import concourse.mybir as mybir
    import concourse.tile as tile
    from concourse import bacc
    from concourse.masks import make_identity

    dt = mybir.dt
    nc = bacc.Bacc("TRN2", target_bir_lowering=False, debug=False, num_devices=8)

    vis = nc.dram_tensor("vis", (N_VIS + 1, D), dt.float32, kind="ExternalInput").ap()
    idx = nc.dram_tensor("idx", (T, 1), dt.int32, kind="ExternalInput").ap()
    wqkv = nc.dram_tensor("wqkv", (DEPTH, D, NH * 3 * P), dt.bfloat16, kind="ExternalInput").ap()
    bqkv = nc.dram_tensor("bqkv", (DEPTH, NH * 3, P), dt.float32, kind="ExternalInput").ap()
    wo = nc.dram_tensor("wo", (DEPTH, NH * P, D), dt.bfloat16, kind="ExternalInput").ap()
    bwo = nc.dram_tensor("bwo", (DEPTH, KD, P), dt.float32, kind="ExternalInput").ap()
    w1 = nc.dram_tensor("w1", (DEPTH, D, HID), dt.bfloat16, kind="ExternalInput").ap()
    b1 = nc.dram_tensor("b1", (DEPTH, HB, P), dt.float32, kind="ExternalInput").ap()
    w2 = nc.dram_tensor("w2", (DEPTH, HID, D), dt.bfloat16, kind="ExternalInput").ap()
    b2 = nc.dram_tensor("b2", (DEPTH, KD, P), dt.float32, kind="ExternalInput").ap()
    wdec = nc.dram_tensor("wdec", (D, D), dt.bfloat16, kind="ExternalInput").ap()
    bdec = nc.dram_tensor("bdec", (KD, P), dt.float32, kind="ExternalInput").ap()
    xidx = nc.dram_tensor("xidx", (TQ, 1), dt.int32, kind="ExternalInput").ap()
    y = nc.dram_tensor("y", (TQ, D), dt.float32, kind="ExternalOutput").ap()

    with tile.TileContext(nc) as tc:
        from contextlib import ExitStack
        ctx = ExitStack()
        with ctx:
            const = ctx.enter_context(tc.tile_pool(name="const", bufs=1))
            xp = ctx.enter_context(tc.tile_pool(name="xp", bufs=1))
            xlnp = ctx.enter_context(tc.tile_pool(name="xlnp", bufs=1))
            qkvp = ctx.enter_context(tc.tile_pool(name="qkvp", bufs=2))
            vtp = ctx.enter_context(tc.tile_pool(name="vtp", bufs=2))
            ptp = ctx.enter_context(tc.tile_pool(name="ptp", bufs=2))
            aop = ctx.enter_context(tc.tile_pool(name="aop", bufs=1))
            hp = ctx.enter_context(tc.tile_pool(name="hp", bufs=1))
            wqp = ctx.enter_context(tc.tile_pool(name="wqp", bufs=2))
            wop = ctx.enter_context(tc.tile_pool(name="wop", bufs=1))
            w1p = ctx.enter_context(tc.tile_pool(name="w1p", bufs=2))
            w2p = ctx.enter_context(tc.tile_pool(name="w2p", bufs=3))
            tokp = ctx.enter_context(tc.tile_pool(name="tokp", bufs=2))
            scr = ctx.enter_context(tc.tile_pool(name="scr", bufs=2))
            xcp = ctx.enter_context(tc.tile_pool(name="xcp", bufs=2))
            statp = ctx.enter_context(tc.tile_pool(name="statp", bufs=1))
            dnp = ctx.enter_context(tc.tile_pool(name="dnp", bufs=2))
            idxp = ctx.enter_context(tc.tile_pool(name="idxp", bufs=2))
            biasp = ctx.enter_context(tc.tile_pool(name="biasp", bufs=1))

            drp = ctx.enter_context(tc.tile_pool(name="drp", bufs=1, space="DRAM"))
            psA = ctx.enter_context(tc.tile_pool(name="psA", bufs=3, space="PSUM"))
            psC = ctx.enter_context(tc.tile_pool(name="psC", bufs=2, space="PSUM"))
            psD = ctx.enter_context(tc.tile_pool(name="psD", bufs=3, space="PSUM"))

            f32 = dt.float32
            bf = dt.bfloat16
            f16 = dt.float16

            ones_kx1 = const.tile([P, 1], f32, tag="ones_kx1")
            nc.any.memset(ones_kx1[:], 1.0)
            ones_kx1h = const.tile([P, 1], f16, tag="ones_kx1h")
            nc.any.memset(ones_kx1h[:], 1.0)
            ones_1xp_t = const.tile([1, P], f32, tag="ones_1xp")
            nc.any.memset(ones_1xp_t[:], 1.0)
            ones_1xp = ones_1xp_t[0:1, :]
            ident_bf = const.tile([P, P], bf, tag="ident_bf")
            make_identity(nc, ident_bf[:])
            ident_f32 = const.tile([P, P], f32, tag="ident_f32")
            make_identity(nc, ident_f32[:])
            eps_t = const.tile([1, 1], f32, tag="eps")
            nc.any.memset(eps_t[:], EPS)
            ident_f16 = const.tile([P, P], f16, tag="ident_f16")
            make_identity(nc, ident_f16[:])

            # persistent activations
            x = xp.tile([P, KD, T], f16, tag="x")
            xln = xlnp.tile([P, KD, T], bf, tag="xln")
            aout = aop.tile([P, NH, TQ], bf, tag="aout")

            # ---- gather + transpose input: x0^T ----
            for tb in range(T // P):
                it = idxp.tile([P, 1], dt.int32, tag="idx")
                nc.sync.dma_start(it[:], idx[tb * P:(tb + 1) * P, :])
                gx = tokp.tile([P, D], f32, tag="tok")
                nc.gpsimd.indirect_dma_start(
                    out=gx[:], out_offset=None, in_=vis[:],
                    in_offset=bass.IndirectOffsetOnAxis(ap=it[:, 0:1], axis=0),
                )
                for kt in range(KD):
                    pst = psA.tile([P, P], f32, tag="a")
                    nc.tensor.transpose(pst[:], gx[:, kt * P:(kt + 1) * P], ident_f32[:])
                    nc.vector.tensor_copy(x[:, kt, tb * P:(tb + 1) * P], pst[:])

            def layer_norm(src, c0, c1, gevict):
                """LN over feature dim of src [P, KD, T] for token range [c0, c1).
                gevict(kt, cs, ce, xc_ap, r_ap) consumes normalized output."""
                # all stats at base partition 0, free-dim segments per chunk:
                # seg 0=s, 1=s2->m2, 2=mu, 3=var->std, 4=mu^2->r
                for cs in range(c0, c1, CH):
                    st = statp.tile([1, 5 * CH], f32, tag="stats")
                    sg = lambda i: st[0:1, i * CH:(i + 1) * CH]
                    ps = psC.tile([P, CH], f32, tag="c")
                    for kt in range(KD):
                        nc.tensor.matmul(ps[0:1, :], ones_kx1h[:], x[:, kt, cs:cs + CH],
                                         start=(kt == 0), stop=(kt == KD - 1))
                    nc.vector.tensor_copy(sg(0), ps[0:1, :])
                    ps2 = psC.tile([P, CH], f32, tag="c")
                    for kt in range(KD):
                        sq = scr.tile([P, CH], f32, tag="scr")
                        nc.vector.tensor_mul(sq[:], x[:, kt, cs:cs + CH], x[:, kt, cs:cs + CH])
                        nc.tensor.matmul(ps2[0:1, :], ones_kx1[:], sq[:],
                                         start=(kt == 0), stop=(kt == KD - 1))
                    nc.vector.tensor_copy(sg(1), ps2[0:1, :])
                    nc.vector.tensor_scalar_mul(sg(2), sg(0), 1.0 / D)       # mu
                    nc.vector.tensor_scalar_mul(sg(1), sg(1), 1.0 / D)       # E[x^2]
                    nc.vector.tensor_mul(sg(4), sg(2), sg(2))                # mu^2
                    nc.vector.tensor_sub(sg(3), sg(1), sg(4))                # var
                    nc.scalar.activation(sg(3), sg(3),
                                         mybir.ActivationFunctionType.Sqrt,
                                         bias=eps_t[0:1, 0:1])
                    nc.vector.reciprocal(sg(4), sg(3))                       # r
                    pmu = psC.tile([P, CH], f32, tag="c")
                    nc.tensor.matmul(pmu[:], ones_1xp, sg(2), start=True, stop=True)
                    pr = psC.tile([P, CH], f32, tag="c")
                    nc.tensor.matmul(pr[:], ones_1xp, sg(4), start=True, stop=True)
                    for kt in range(KD):
                        xc = xcp.tile([P, CH], f32, tag="xc")
                        nc.vector.tensor_sub(xc[:], x[:, kt, cs:cs + CH], pmu[:])
                        gevict(kt, cs, xc, pr)

            def ln_to_xln(kt, cs, xc, pr):
                nc.vector.tensor_mul(xln[:, kt, cs:cs + CH], xc[:], pr[:])

            for l in range(DEPTH):
                # ---------- LN1 (full range: K/V need all tokens) ----------
                layer_norm(x, 0, T, ln_to_xln)

                # qkv biases for this layer
                bq = biasp.tile([P, NH * 3], f32, tag="bq")
                nc.sync.dma_start(bq[:], bqkv[l].rearrange("a p -> p a"))
                bo_t = biasp.tile([P, KD], f32, tag="bo")
                nc.sync.dma_start(bo_t[:], bwo[l].rearrange("a p -> p a"))
                b1_t = biasp.tile([P, HB], f32, tag="b1")
                nc.sync.dma_start(b1_t[:], b1[l].rearrange("a p -> p a"))
                b2_t = biasp.tile([P, KD], f32, tag="b2")
                nc.sync.dma_start(b2_t[:], b2[l].rearrange("a p -> p a"))

                wot = wop.tile([P, NH, D], bf, tag="wo")
                for kb in range(NH):
                    nc.sync.dma_start(wot[:, kb, :], wo[l, kb * P:(kb + 1) * P, :])

                q0 = T - TQ                     # own-half query range


                # ---------- attention, head-by-head ----------
                for h in range(NH):
                    wqt = wqp.tile([P, KD, 3 * P], bf, tag="wq")
                    for kt in range(KD):
                        nc.sync.dma_start(
                            wqt[:, kt, :],
                            wqkv[l, kt * P:(kt + 1) * P, h * 3 * P:(h + 1) * 3 * P])
                    qkvh = qkvp.tile([P, 3, T], bf, tag="qkvh")
                    for m in range(3):
                        m0 = q0 if m == 0 else 0
                        for cs in range(m0, T, CH):
                            ps = psA.tile([P, CH], f32, tag="a")
                            for kt in range(KD):
                                nc.tensor.matmul(ps[:], wqt[:, kt, m * P:(m + 1) * P],
                                                 xln[:, kt, cs:cs + CH],
                                                 start=(kt == 0), stop=(kt == KD - 1))
                            nc.vector.tensor_scalar_add(qkvh[:, m, cs:cs + CH], ps[:],
                                                        bq[:, h * 3 + m:h * 3 + m + 1])
                    # transpose V (and its ones-row) -> vaug [T, 128]
                    vaug = vtp.tile([P, T // P, P], bf, tag="vaug")
                    for tb in range(T // P):
                        pst = psA.tile([P, P], bf, tag="a")
                        nc.tensor.transpose(pst[:], qkvh[:, 2, tb * P:(tb + 1) * P],
                                            ident_bf[:])
                        nc.vector.tensor_copy(vaug[:, tb, :], pst[:])
                    # scores^T -> exp -> PV, per query chunk, in 2 half-passes of Tk
                    pt = ptp.tile([P, 8, CH], bf, tag="pt")
                    for cs in range(q0, T, CH):
                        pv = psD.tile([P, CH], f32, tag="d")
                        for half in range(2):
                            for tb8 in range(8):
                                tb = half * 8 + tb8
                                ps = psA.tile([P, CH], f32, tag="a")
                                nc.tensor.matmul(ps[:], qkvh[:, 1, tb * P:(tb + 1) * P],
                                                 qkvh[:, 0, cs:cs + CH],
                                                 start=True, stop=True)
                                nc.scalar.activation(pt[:, tb8, :], ps[:],
                                                     mybir.ActivationFunctionType.Exp)
                            for tb8 in range(8):
                                tb = half * 8 + tb8
                                nc.tensor.matmul(pv[:], vaug[:, tb, :], pt[:, tb8, :],
                                                 start=(tb == 0), stop=(tb == T // P - 1))
                        # normalize by denominator (row 96 of pv)
                        dn = dnp.tile([1, CH], f32, tag="dn")
                        nc.vector.tensor_copy(dn[:], pv[DH:DH + 1, :])
                        pc = psC.tile([P, CH], f32, tag="c")
                        nc.tensor.matmul(pc[:], ones_1xp, dn[:], start=True, stop=True)
                        rc = scr.tile([P, CH], f32, tag="scr")
                        nc.vector.reciprocal(rc[:], pc[:])
                        nc.vector.tensor_mul(aout[:, h, cs - q0:cs - q0 + CH], pv[:], rc[:])

                # ---------- Wo + residual ----------
                for cs in range(q0, T, CH):
                    for m in range(KD):
                        ps = psA.tile([P, CH], f32, tag="a")
                        for kb in range(NH):
                            nc.tensor.matmul(ps[:], wot[:, kb, m * P:(m + 1) * P],
                                             aout[:, kb, cs - q0:cs - q0 + CH],
                                             start=(kb == 0), stop=(kb == NH - 1))
                        t = scr.tile([P, CH], f32, tag="scr")
                        nc.scalar.activation(t[:], ps[:],
                                             mybir.ActivationFunctionType.Identity,
                                             bias=bo_t[:, m:m + 1])
                        nc.vector.tensor_add(x[:, m, cs:cs + CH], x[:, m, cs:cs + CH], t[:])

                # ---------- LN2 + FFN + residual ----------
                f0 = T - TQ
                layer_norm(x, f0, T, ln_to_xln)
                for cs in range(f0, T, CH):
                    ht = hp.tile([P, HB, CH], bf, tag="h")
                    for mg in range(8):
                        w1t = w1p.tile([P, KD, 3 * P], bf, tag="w1")
                        for kt in range(KD):
                            nc.sync.dma_start(
                                w1t[:, kt, :],
                                w1[l, kt * P:(kt + 1) * P, mg * 3 * P:(mg + 1) * 3 * P])
                        for hbl in range(3):
                            hb = mg * 3 + hbl
                            ph = psA.tile([P, CH], f32, tag="a")
                            for kt in range(KD):
                                nc.tensor.matmul(ph[:], w1t[:, kt, hbl * P:(hbl + 1) * P],
                                                 xln[:, kt, cs:cs + CH],
                                                 start=(kt == 0), stop=(kt == KD - 1))
                            nc.scalar.activation(ht[:, hb, :], ph[:],
                                                 mybir.ActivationFunctionType.Gelu,
                                                 bias=b1_t[:, hb:hb + 1])
                    for mh in range(2):
                        pds = [psD.tile([P, CH], f32, tag="d", name=f"pd{_i}") for _i in range(3)]
                        for kb in range(HB):
                            w2t = w2p.tile([P, 3 * P], bf, tag="w2")
                            nc.sync.dma_start(w2t[:],
                                              w2[l, kb * P:(kb + 1) * P,
                                                 mh * 3 * P:(mh + 1) * 3 * P])
                            for m3 in range(3):
                                nc.tensor.matmul(pds[m3][:], w2t[:, m3 * P:(m3 + 1) * P],
                                                 ht[:, kb, :],
                                                 start=(kb == 0), stop=(kb == HB - 1))
                        for m3 in range(3):
                            m = mh * 3 + m3
                            t = scr.tile([P, CH], f32, tag="scr")
                            nc.scalar.activation(t[:], pds[m3][:],
                                                 mybir.ActivationFunctionType.Identity,
                                                 bias=b2_t[:, m:m + 1])
                            nc.vector.tensor_add(x[:, m, cs:cs + CH],
                                                 x[:, m, cs:cs + CH], t[:])

                if l == 0:
                    # exchange x1 halves within batch pairs
                    cc_in = drp.tile([TQ, D], f16, tag="cc_in")
                    cc_out = drp.tile([2 * TQ, D], f16, tag="cc_out")
                    for tb in range(TQ // P):
                        tk16 = tokp.tile([P, D], f16, tag="tok16")
                        for kt in range(KD):
                            pst = psA.tile([P, P], f16, tag="a")
                            nc.tensor.transpose(
                                pst[:], x[:, kt, q0 + tb * P:q0 + (tb + 1) * P],
                                ident_f16[:])
                            nc.vector.tensor_copy(tk16[:, kt * P:(kt + 1) * P], pst[:])
                        nc.sync.dma_start(cc_in[tb * P:(tb + 1) * P, :], tk16[:])
                    nc.gpsimd.collective_compute(
                        "AllGather",
                        mybir.AluOpType.bypass,
                        replica_groups=[[0, 1], [2, 3], [4, 5], [6, 7]],
                        ins=[cc_in.opt()],
                        outs=[cc_out.opt()],
                    )
                    for tb in range(TQ // P):
                        xit = idxp.tile([P, 1], dt.int32, tag="idx")
                        nc.sync.dma_start(xit[:], xidx[tb * P:(tb + 1) * P, :])
                        g16 = tokp.tile([P, D], f16, tag="tok16")
                        nc.gpsimd.indirect_dma_start(
                            out=g16[:], out_offset=None, in_=cc_out[:],
                            in_offset=bass.IndirectOffsetOnAxis(ap=xit[:, 0:1], axis=0),
                        )
                        for kt in range(KD):
                            pst = psA.tile([P, P], f16, tag="a")
                            nc.tensor.transpose(pst[:], g16[:, kt * P:(kt + 1) * P],
                                                ident_f16[:])
                            nc.vector.tensor_copy(x[:, kt, tb * P:(tb + 1) * P], pst[:])

            # ---------- final LN + decoder head + transpose out ----------
            layer_norm(x, T - TQ, T, ln_to_xln)
            wdt = w1p.tile([P, KD, 3 * P], bf, tag="w1")  # share slot tag with w1
            wdt2 = w1p.tile([P, KD, 3 * P], bf, tag="w1")
            for kt in range(KD):
                nc.sync.dma_start(wdt[:, kt, :], wdec[kt * P:(kt + 1) * P, 0:3 * P])
                nc.sync.dma_start(wdt2[:, kt, :], wdec[kt * P:(kt + 1) * P, 3 * P:6 * P])
            bd_t = biasp.tile([P, KD], f32, tag="bd")
            nc.sync.dma_start(bd_t[:], bdec.rearrange("a p -> p a"))
            yT = hp.tile([P, KD, CH], f32, tag="h")
            for cs in range(T - TQ, T, CH):
                for mh in range(2):
                    wsel = wdt if mh == 0 else wdt2
                    pds = [psD.tile([P, CH], f32, tag="d", name=f"pd{_i}") for _i in range(3)]
                    for m3 in range(3):
                        for kt in range(KD):
                            nc.tensor.matmul(pds[m3][:], wsel[:, kt, m3 * P:(m3 + 1) * P],
                                             xln[:, kt, cs:cs + CH],
                                             start=(kt == 0), stop=(kt == KD - 1))
                        m = mh * 3 + m3
                        nc.scalar.activation(yT[:, m, :], pds[m3][:],
                                             mybir.ActivationFunctionType.Identity,
                                             bias=bd_t[:, m:m + 1])
                for ts in range(CH // P):
                    ytok = tokp.tile([P, D], f32, tag="tok")
                    for m in range(KD):
                        pst = psA.tile([P, P], f32, tag="a")
                        nc.tensor.transpose(pst[:], yT[:, m, ts * P:(ts + 1) * P],
                                            ident_f32[:])
                        nc.vector.tensor_copy(ytok[:, m * P:(m + 1) * P], pst[:])
                    t0 = cs - (T - TQ) + ts * P
                    nc.sync.dma_start(y[t0:t0 + P, :], ytok[:])

    nc.compile()
    return nc


def _prep_weights(inputs):
    """Host-side weight folding/packing. Returns dict of shared arrays."""
    g1, be1 = inputs["gamma1"], inputs["beta1"]
    g2, be2 = inputs["gamma2"], inputs["beta2"]
    Wqkv, bqkv = inputs["Wqkv"], inputs["bqkv"]
    Wo, bo = inputs["Wo"], inputs["bo"]
    W1, b1 = inputs["W1"], inputs["b1"]
    W2, b2 = inputs["W2"], inputs["b2"]
    gn, gb = inputs["gn"], inputs["gb"]
    Wdec, bdec = inputs["Wdec"], inputs["bdec"]

    wqkv_a = np.zeros((DEPTH, D, NH * 3 * P), np.float32)
    bqkv_a = np.zeros((DEPTH, NH * 3, P), np.float32)
    wo_a = np.zeros((DEPTH, NH * P, D), np.float32)
    bwo_a = np.zeros((DEPTH, KD, P), np.float32)
    w1_a = np.zeros((DEPTH, D, HID), np.float32)
    b1_a = np.zeros((DEPTH, HB, P), np.float32)
    w2_a = np.zeros((DEPTH, HID, D), np.float32)
    b2_a = np.zeros((DEPTH, KD, P), np.float32)
    scale = 1.0 / np.sqrt(DH)
    for l in range(DEPTH):
        Wp = Wqkv[l] * g1[l][None, :]                  # fold gamma1
        bp = bqkv[l] + Wqkv[l] @ be1[l]                # fold beta1
        Wp = Wp.copy()
        bp = bp.copy()
        Wp[:D] *= scale                                # fold 1/sqrt(dh) into Q
        bp[:D] *= scale
        for h in range(NH):
            for c in range(3):                         # q,k,v
                rows = slice(c * D + h * DH, c * D + (h + 1) * DH)
                wqkv_a[l, :, (h * 3 + c) * P:(h * 3 + c) * P + DH] = Wp[rows].T
                bqkv_a[l, h * 3 + c, :DH] = bp[rows]
            bqkv_a[l, h * 3 + 2, DH] = 1.0             # ones-row -> denominators
            wo_a[l, h * P:h * P + DH, :] = Wo[l][:, h * DH:(h + 1) * DH].T
        bwo_a[l] = bo[l].reshape(KD, P)
        w1_a[l] = (W1[l] * g2[l][None, :]).T
        b1_a[l] = (b1[l] + W1[l] @ be2[l]).reshape(HB, P)
        w2_a[l] = W2[l].T
        b2_a[l] = b2[l].reshape(KD, P)
    wdec_a = (Wdec * gn[None, :]).T
    bdec_a = (bdec + Wdec @ gb).reshape(KD, P)
    return {
        "wqkv": wqkv_a.astype(BF16), "bqkv": bqkv_a,
        "wo": wo_a.astype(BF16), "bwo": bwo_a,
        "w1": w1_a.astype(BF16), "b1": b1_a,
        "w2": w2_a.astype(BF16), "b2": b2_a,
        "wdec": wdec_a.astype(BF16), "bdec": bdec_a,
    }


def kernel(**inputs):
    from concourse.bass_utils import run_bass_kernel_spmd

    inputs = {k: np.asarray(v) for k, v in inputs.items()}
    if "nc" not in _cache:
        _cache["nc"] = _build()
    nc = _cache["nc"]

    shared = _prep_weights(inputs)
    mask = inputs["mask"]
    vt = inputs["visible_tokens"].astype(np.float32)
    mt = inputs["mask_token"].astype(np.float32)

    nv = np.clip(np.cumsum(mask.astype(np.int64), axis=1) - 1, 0, N_VIS - 1)
    idx_full = np.where(mask, nv, N_VIS).astype(np.int32)     # row 512 = mask token

    in_maps = []
    for core in range(8):
        b, s = core // 2, core % 2
        if s == 0:
            perm = np.concatenate([np.arange(TQ, T), np.arange(0, TQ)])
        else:
            perm = np.arange(T)
        vis_ext = np.concatenate([vt[b], mt[None, :]], axis=0)
        m = dict(shared)
        m["vis"] = np.ascontiguousarray(vis_ext)
        m["idx"] = np.ascontiguousarray(idx_full[b][perm][:, None])
        m["xidx"] = np.ascontiguousarray(perm[:TQ].astype(np.int32)[:, None])
        in_maps.append(m)

    res = run_bass_kernel_spmd(nc, in_maps, core_ids=list(range(8)),
                               **_cache.get("run_kwargs", {}))
    _cache["last_results"] = res

    out = np.zeros((B, T, D), np.float32)
    for core in range(8):
        b, s = core // 2, core % 2
        out[b, s * TQ:(s + 1) * TQ] = res.results[core]["y"]
    return out


if __name__ == "__main__":
    rng = np.random.default_rng(0)
    print("building...")
    _build()
    print("built ok")


# revision 18
# speedup vs baseline: 1.5438x; 1.0424x over previous
"""Trainium2 Bass kernel for nn_DecoderHead (MAE-style decoder head).

Strategy (8 NeuronCores): data-parallel over batch B=4 x 2-way token split
per batch. Cores sharing a batch both compute layer 0 for all 2048 tokens
(cheaper than cross-core K/V exchange), then layer 1 + head for their own
1024-token half only. Tokens are permuted host-side so every core's "own"
half sits at positions 1024..2048 -- attention is permutation-equivariant,
so one NEFF serves all cores.

On-device layout is feature-major (x^T: [D, T] with D on partitions), which
makes every linear a plain lhsT.T @ rhs chain with host-pre-transposed
weights and no activation transposes. Heads are padded 96->128 so per-head
Q/K/V slices are partition-aligned; a ones-row injected into V (via the
padded bias) makes the PV matmul emit softmax denominators for free.
Softmax skips max-subtraction (|scores| <= ~2 by construction). LN
gamma/beta are folded into the adjacent weights host-side; LN stats are
computed with ones-vector matmuls on the PE and broadcast across partitions
with rank-1 matmuls.
"""

import sys
import numpy as np

sys.path.insert(0, "/opt/trn_rl_repo")

import ml_dtypes

P = 128
B = 4
N_VIS = 512
T = 2048          # N_TOT
D = 768
KD = D // P       # 6
NH = 8
DH = 96
HID = 3072
HB = HID // P     # 24
DEPTH = 2
TQ = 1024         # own-half tokens per core
CH = 512          # token chunk
EPS = 1e-5

BF16 = ml_dtypes.bfloat16

_cache = {}


def _build():
    import concourse.bass as bass
    import concourse.mybir as mybir
    import concourse.tile as tile
    from concourse import bacc
    from concourse.masks import make_identity

    dt = mybir.dt
    nc = bacc.Bacc("TRN2", target_bir_lowering=False, debug=False, num_devices=8)

    vis = nc.dram_tensor("vis", (N_VIS + 1, D), dt.float32, kind="ExternalInput").ap()
    idx = nc.dram_tensor("idx", (T, 1), dt.int32, kind="ExternalInput").ap()
    wqkv = nc.dram_tensor("wqkv", (DEPTH, D, NH * 3 * P), dt.bfloat16, kind="ExternalInput").ap()
    bqkv = nc.dram_tensor("bqkv", (DEPTH, NH * 3, P), dt.float32, kind="ExternalInput").ap()
    wo = nc.dram_tensor("wo", (DEPTH, NH * P, D), dt.bfloat16, kind="ExternalInput").ap()
    bwo = nc.dram_tensor("bwo", (DEPTH, KD, P), dt.float32, kind="ExternalInput").ap()
    w1 = nc.dram_tensor("w1", (DEPTH, D, HID), dt.bfloat16, kind="ExternalInput").ap()
    b1 = nc.dram_tensor("b1", (DEPTH, HB, P), dt.float32, kind="ExternalInput").ap()
    w2 = nc.dram_tensor("w2", (DEPTH, HID, D), dt.bfloat16, kind="ExternalInput").ap()
    b2 = nc.dram_tensor("b2", (DEPTH, KD, P), dt.float32, kind="ExternalInput").ap()
    wdec = nc.dram_tensor("wdec", (D, D), dt.bfloat16, kind="ExternalInput").ap()
    bdec = nc.dram_tensor("bdec", (KD, P), dt.float32, kind="ExternalInput").ap()
    xidx = nc.dram_tensor("xidx", (TQ, 1), dt.int32, kind="ExternalInput").ap()
    y = nc.dram_tensor("y", (TQ, D), dt.float32, kind="ExternalOutput").ap()

    with tile.TileContext(nc) as tc:
        from contextlib import ExitStack
        ctx = ExitStack()
        with ctx:
            const = ctx.enter_context(tc.tile_pool(name="const", bufs=1))
            xp = ctx.enter_context(tc.tile_pool(name="xp", bufs=1))
            xlnp = ctx.enter_context(tc.tile_pool(name="xlnp", bufs=1))
            qkvp = ctx.enter_context(tc.tile_pool(name="qkvp", bufs=2))
            vtp = ctx.enter_context(tc.tile_pool(name="vtp", bufs=2))
            ptp = ctx.enter_context(tc.tile_pool(name="ptp", bufs=2))
            aop = ctx.enter_context(tc.tile_pool(name="aop", bufs=1))
            hp = ctx.enter_context(tc.tile_pool(name="hp", bufs=1))
            wqp = ctx.enter_context(tc.tile_pool(name="wqp", bufs=2))
            wop = ctx.enter_context(tc.tile_pool(name="wop", bufs=1))
            w1p = ctx.enter_context(tc.tile_pool(name="w1p", bufs=2))
            w2p = ctx.enter_context(tc.tile_pool(name="w2p", bufs=3))
            tokp = ctx.enter_context(tc.tile_pool(name="tokp", bufs=2))
            scr = ctx.enter_context(tc.tile_pool(name="scr", bufs=2))
            xcp = ctx.enter_context(tc.tile_pool(name="xcp", bufs=2))
            statp = ctx.enter_context(tc.tile_pool(name="statp", bufs=1))
            dnp = ctx.enter_context(tc.tile_pool(name="dnp", bufs=2))
            idxp = ctx.enter_context(tc.tile_pool(name="idxp", bufs=2))
            biasp = ctx.enter_context(tc.tile_pool(name="biasp", bufs=1))

            drp = ctx.enter_context(tc.tile_pool(name="drp", bufs=1, space="DRAM"))
            psA = ctx.enter_context(tc.tile_pool(name="psA", bufs=3, space="PSUM"))
            psC = ctx.enter_context(tc.tile_pool(name="psC", bufs=2, space="PSUM"))
            psD = ctx.enter_context(tc.tile_pool(name="psD", bufs=3, space="PSUM"))

            f32 = dt.float32
            bf = dt.bfloat16
            f16 = dt.float16

            ones_kx1 = const.tile([P, 1], f32, tag="ones_kx1")
            nc.any.memset(ones_kx1[:], 1.0)
            ones_kx1h = const.tile([P, 1], f16, tag="ones_kx1h")
            nc.any.memset(ones_kx1h[:], 1.0)
            ones_1xp_t = const.tile([1, P], f32, tag="ones_1xp")
            nc.any.memset(ones_1xp_t[:], 1.0)
            ones_1xp = ones_1xp_t[0:1, :]
            ident_bf = const.tile([P, P], bf, tag="ident_bf")
            make_identity(nc, ident_bf[:])
            ident_f32 = const.tile([P, P], f32, tag="ident_f32")
            make_identity(nc, ident_f32[:])
            eps_t = const.tile([1, 1], f32, tag="eps")
            nc.any.memset(eps_t[:], EPS)
            ident_f16 = const.tile([P, P], f16, tag="ident_f16")
            make_identity(nc, ident_f16[:])

            # persistent activations
            x = xp.tile([P, KD, T], f16, tag="x")
            xln = xlnp.tile([P, KD, T], bf, tag="xln")
            aout = aop.tile([P, NH, TQ], bf, tag="aout")

            # ---- gather + transpose input: x0^T ----
            for tb in range(T // P):
                it = idxp.tile([P, 1], dt.int32, tag="idx")
                nc.sync.dma_start(it[:], idx[tb * P:(tb + 1) * P, :])
                gx = tokp.tile([P, D], f32, tag="tok")
                nc.gpsimd.indirect_dma_start(
                    out=gx[:], out_offset=None, in_=vis[:],
                    in_offset=bass.IndirectOffsetOnAxis(ap=it[:, 0:1], axis=0),
                )
                for kt in range(KD):
                    pst = psA.tile([P, P], f32, tag="a")
                    nc.tensor.transpose(pst[:], gx[:, kt * P:(kt + 1) * P], ident_f32[:])
                    nc.vector.tensor_copy(x[:, kt, tb * P:(tb + 1) * P], pst[:])

            def layer_norm(src, c0, c1, gevict):
                """LN over feature dim of src [P, KD, T] for token range [c0, c1).
                gevict(kt, cs, ce, xc_ap, r_ap) consumes normalized output."""
                # all stats at base partition 0, free-dim segments per chunk:
                # seg 0=s, 1=s2->m2, 2=mu, 3=var->std, 4=mu^2->r
                for cs in range(c0, c1, CH):
                    st = statp.tile([1, 5 * CH], f32, tag="stats")
                    sg = lambda i: st[0:1, i * CH:(i + 1) * CH]
                    ps = psA.tile([P, CH], f32, tag="a")
                    for kt in range(KD):
                        nc.tensor.matmul(ps[0:1, :], ones_kx1h[:], x[:, kt, cs:cs + CH],
                                         start=(kt == 0), stop=(kt == KD - 1))
                    nc.vector.tensor_copy(sg(0), ps[0:1, :])
                    ps2 = psA.tile([P, CH], f32, tag="a")
                    for kt in range(KD):
                        sq = scr.tile([P, CH], f32, tag="scr")
                        nc.vector.tensor_mul(sq[:], x[:, kt, cs:cs + CH], x[:, kt, cs:cs + CH])
                        nc.tensor.matmul(ps2[0:1, :], ones_kx1[:], sq[:],
                                         start=(kt == 0), stop=(kt == KD - 1))
                    nc.vector.tensor_copy(sg(1), ps2[0:1, :])
                    nc.vector.tensor_scalar_mul(sg(2), sg(0), 1.0 / D)       # mu
                    nc.vector.tensor_scalar_mul(sg(1), sg(1), 1.0 / D)       # E[x^2]
                    nc.vector.tensor_mul(sg(4), sg(2), sg(2))                # mu^2
                    nc.vector.tensor_sub(sg(3), sg(1), sg(4))                # var
                    nc.scalar.activation(sg(3), sg(3),
                                         mybir.ActivationFunctionType.Sqrt,
                                         bias=eps_t[0:1, 0:1])
                    nc.vector.reciprocal(sg(4), sg(3))                       # r
                    pmu = psC.tile([P, CH], f32, tag="c")
                    nc.tensor.matmul(pmu[:], ones_1xp, sg(2), start=True, stop=True)
                    pr = psC.tile([P, CH], f32, tag="c")
                    nc.tensor.matmul(pr[:], ones_1xp, sg(4), start=True, stop=True)
                    for kt in range(KD):
                        xc = xcp.tile([P, CH], f32, tag="xc")
                        nc.vector.tensor_sub(xc[:], x[:, kt, cs:cs + CH], pmu[:])
                        gevict(kt, cs, xc, pr)

            def ln_to_xln(kt, cs, xc, pr):
                nc.vector.tensor_mul(xln[:, kt, cs:cs + CH], xc[:], pr[:])

            for l in range(DEPTH):
                # ---------- LN1 (full range: K/V need all tokens) ----------
                layer_norm(x, 0, T, ln_to_xln)

                # qkv biases for this layer
                bq = biasp.tile([P, NH * 3], f32, tag="bq")
                nc.sync.dma_start(bq[:], bqkv[l].rearrange("a p -> p a"))
                bo_t = biasp.tile([P, KD], f32, tag="bo")
                nc.sync.dma_start(bo_t[:], bwo[l].rearrange("a p -> p a"))
                b1_t = biasp.tile([P, HB], f32, tag="b1")
                nc.sync.dma_start(b1_t[:], b1[l].rearrange("a p -> p a"))
                b2_t = biasp.tile([P, KD], f32, tag="b2")
                nc.sync.dma_start(b2_t[:], b2[l].rearrange("a p -> p a"))

                wot = wop.tile([P, NH, D], bf, tag="wo")
                for kb in range(NH):
                    nc.sync.dma_start(wot[:, kb, :], wo[l, kb * P:(kb + 1) * P, :])

                q0 = T - TQ                     # own-half query range


                # ---------- attention, head-by-head ----------
                for h in range(NH):
                    wqt = wqp.tile([P, KD, 3 * P], bf, tag="wq")
                    for kt in range(KD):
                        nc.sync.dma_start(
                            wqt[:, kt, :],
                            wqkv[l, kt * P:(kt + 1) * P, h * 3 * P:(h + 1) * 3 * P])
                    qkvh = qkvp.tile([P, 3, T], bf, tag="qkvh")
                    for m in range(3):
                        m0 = q0 if m == 0 else 0
                        for cs in range(m0, T, CH):
                            ps = psA.tile([P, CH], f32, tag="a")
                            for kt in range(KD):
                                nc.tensor.matmul(ps[:], wqt[:, kt, m * P:(m + 1) * P],
                                                 xln[:, kt, cs:cs + CH],
                                                 start=(kt == 0), stop=(kt == KD - 1))
                            nc.vector.tensor_scalar_add(qkvh[:, m, cs:cs + CH], ps[:],
                                                        bq[:, h * 3 + m:h * 3 + m + 1])
                    # transpose V (and its ones-row) -> vaug [T, 128]
                    vaug = vtp.tile([P, T // P, P], bf, tag="vaug")
                    for tb in range(T // P):
                        pst = psA.tile([P, P], bf, tag="a")
                        nc.tensor.transpose(pst[:], qkvh[:, 2, tb * P:(tb + 1) * P],
                                            ident_bf[:])
                        nc.vector.tensor_copy(vaug[:, tb, :], pst[:])
                    # scores^T -> exp -> PV, per query chunk, in 2 half-passes of Tk
                    pt = ptp.tile([P, 8, CH], bf, tag="pt")
                    for cs in range(q0, T, CH):
                        pv = psD.tile([P, CH], f32, tag="d")
                        for half in range(2):
                            for tb8 in range(8):
                                tb = half * 8 + tb8
                                ps = psA.tile([P, CH], f32, tag="a")
                                nc.tensor.matmul(ps[:], qkvh[:, 1, tb * P:(tb + 1) * P],
                                                 qkvh[:, 0, cs:cs + CH],
                                                 start=True, stop=True)
                                nc.scalar.activation(pt[:, tb8, :], ps[:],
                                                     mybir.ActivationFunctionType.Exp)
                            for tb8 in range(8):
                                tb = half * 8 + tb8
                                nc.tensor.matmul(pv[:], vaug[:, tb, :], pt[:, tb8, :],
                                                 start=(tb == 0), stop=(tb == T // P - 1))
                        # normalize by denominator (row 96 of pv)
                        dn = dnp.tile([1, CH], f32, tag="dn")
                        nc.vector.tensor_copy(dn[:], pv[DH:DH + 1, :])
                        pc = psC.tile([P, CH], f32, tag="c")
                        nc.tensor.matmul(pc[:], ones_1xp, dn[:], start=True, stop=True)
                        rc = scr.tile([P, CH], f32, tag="scr")
                        nc.vector.reciprocal(rc[:], pc[:])
                        nc.vector.tensor_mul(aout[:, h, cs - q0:cs - q0 + CH], pv[:], rc[:])

                # ---------- Wo + residual ----------
                for cs in range(q0, T, CH):
                    for m in range(KD):
                        ps = psA.tile([P, CH], f32, tag="a")
                        for kb in range(NH):
                            nc.tensor.matmul(ps[:], wot[:, kb, m * P:(m + 1) * P],
                                             aout[:, kb, cs - q0:cs - q0 + CH],
                                             start=(kb == 0), stop=(kb == NH - 1))
                        t = scr.tile([P, CH], f32, tag="scr")
                        nc.scalar.activation(t[:], ps[:],
                                             mybir.ActivationFunctionType.Identity,
                                             bias=bo_t[:, m:m + 1])
                        nc.vector.tensor_add(x[:, m, cs:cs + CH], x[:, m, cs:cs + CH], t[:])

                # ---------- LN2 + FFN + residual ----------
                f0 = T - TQ
                layer_norm(x, f0, T, ln_to_xln)
                for cs in range(f0, T, CH):
                    ht = hp.tile([P, HB, CH], bf, tag="h")
                    for mg in range(8):
                        w1t = w1p.tile([P, KD, 3 * P], bf, tag="w1")
                        for kt in range(KD):
                            nc.sync.dma_start(
                                w1t[:, kt, :],
                                w1[l, kt * P:(kt + 1) * P, mg * 3 * P:(mg + 1) * 3 * P])
                        for hbl in range(3):
                            hb = mg * 3 + hbl
                            ph = psA.tile([P, CH], f32, tag="a")
                            for kt in range(KD):
                                nc.tensor.matmul(ph[:], w1t[:, kt, hbl * P:(hbl + 1) * P],
                                                 xln[:, kt, cs:cs + CH],
                                                 start=(kt == 0), stop=(kt == KD - 1))
                            nc.scalar.activation(ht[:, hb, :], ph[:],
                                                 mybir.ActivationFunctionType.Gelu,
                                                 bias=b1_t[:, hb:hb + 1])
                    for mh in range(2):
                        pds = [psD.tile([P, CH], f32, tag="d", name=f"pd{_i}") for _i in range(3)]
                        for kb in range(HB):
                            w2t = w2p.tile([P, 3 * P], bf, tag="w2")
                            nc.sync.dma_start(w2t[:],
                                              w2[l, kb * P:(kb + 1) * P,
                                                 mh * 3 * P:(mh + 1) * 3 * P])
                            for m3 in range(3):
                                nc.tensor.matmul(pds[m3][:], w2t[:, m3 * P:(m3 + 1) * P],
                                                 ht[:, kb, :],
                                                 start=(kb == 0), stop=(kb == HB - 1))
                        for m3 in range(3):
                            m = mh * 3 + m3
                            t = scr.tile([P, CH], f32, tag="scr")
                            nc.scalar.activation(t[:], pds[m3][:],
                                                 mybir.ActivationFunctionType.Identity,
                                                 bias=b2_t[:, m:m + 1])
                            nc.vector.tensor_add(x[:, m, cs:cs + CH],
                                                 x[:, m, cs:cs + CH], t[:])

                if l == 0:
                    # exchange x1 halves within batch pairs, one collective per
                    # 512-token chunk so the first overlaps FFN of chunk 2
                    cc_outs = []
                    for ci in range(TQ // CH):
                        cc_in = drp.tile([CH, D], f16, tag=f"cci{ci}", name=f"cci{ci}")
                        cc_out = drp.tile([2 * CH, D], f16, tag=f"cco{ci}", name=f"cco{ci}")
                        cc_outs.append(cc_out)
                        for tb4 in range(CH // P):
                            tb = ci * (CH // P) + tb4
                            tk16 = tokp.tile([P, D], f16, tag="tok16", name=f"tk{tb}")
                            for kt in range(KD):
                                pst = psA.tile([P, P], f16, tag="a", name=f"pst{tb}_{kt}")
                                nc.tensor.transpose(
                                    pst[:], x[:, kt, q0 + tb * P:q0 + (tb + 1) * P],
                                    ident_f16[:])
                                nc.vector.tensor_copy(tk16[:, kt * P:(kt + 1) * P], pst[:])
                            nc.sync.dma_start(cc_in[tb4 * P:(tb4 + 1) * P, :], tk16[:])
                        nc.gpsimd.collective_compute(
                            "AllGather",
                            mybir.AluOpType.bypass,
                            replica_groups=[[0, 1], [2, 3], [4, 5], [6, 7]],
                            ins=[cc_in.opt()],
                            outs=[cc_out.opt()],
                        )
                    for tb in range(TQ // P):
                        xit = idxp.tile([P, 1], dt.int32, tag="idx", name=f"xit{tb}")
                        nc.sync.dma_start(xit[:], xidx[tb * P:(tb + 1) * P, :])
                        g16 = tokp.tile([P, D], f16, tag="tok16", name=f"g16_{tb}")
                        nc.gpsimd.indirect_dma_start(
                            out=g16[:], out_offset=None,
                            in_=cc_outs[tb // (CH // P)][:],
                            in_offset=bass.IndirectOffsetOnAxis(ap=xit[:, 0:1], axis=0),
                        )
                        for kt in range(KD):
                            pst = psA.tile([P, P], f16, tag="a", name=f"psr{tb}_{kt}")
                            nc.tensor.transpose(pst[:], g16[:, kt * P:(kt + 1) * P],
                                                ident_f16[:])
                            nc.vector.tensor_copy(x[:, kt, tb * P:(tb + 1) * P], pst[:])

            # ---------- final LN + decoder head + transpose out ----------
            layer_norm(x, T - TQ, T, ln_to_xln)
            wdt = w1p.tile([P, KD, 3 * P], bf, tag="w1")  # share slot tag with w1
            wdt2 = w1p.tile([P, KD, 3 * P], bf, tag="w1")
            for kt in range(KD):
                nc.sync.dma_start(wdt[:, kt, :], wdec[kt * P:(kt + 1) * P, 0:3 * P])
                nc.sync.dma_start(wdt2[:, kt, :], wdec[kt * P:(kt + 1) * P, 3 * P:6 * P])
            bd_t = biasp.tile([P, KD], f32, tag="bd")
            nc.sync.dma_start(bd_t[:], bdec.rearrange("a p -> p a"))
            yT = hp.tile([P, KD, CH], f32, tag="h")
            for cs in range(T - TQ, T, CH):
                for mh in range(2):
                    wsel = wdt if mh == 0 else wdt2
                    pds = [psD.tile([P, CH], f32, tag="d", name=f"pd{_i}") for _i in range(3)]
                    for m3 in range(3):
                        for kt in range(KD):
                            nc.tensor.matmul(pds[m3][:], wsel[:, kt, m3 * P:(m3 + 1) * P],
                                             xln[:, kt, cs:cs + CH],
                                             start=(kt == 0), stop=(kt == KD - 1))
                        m = mh * 3 + m3
                        nc.scalar.activation(yT[:, m, :], pds[m3][:],
                                             mybir.ActivationFunctionType.Identity,
                                             bias=bd_t[:, m:m + 1])
                for ts in range(CH // P):
                    ytok = tokp.tile([P, D], f32, tag="tok")
                    for m in range(KD):
                        pst = psA.tile([P, P], f32, tag="a")
                        nc.tensor.transpose(pst[:], yT[:, m, ts * P:(ts + 1) * P],
                                            ident_f32[:])
                        nc.vector.tensor_copy(ytok[:, m * P:(m + 1) * P], pst[:])
                    t0 = cs - (T - TQ) + ts * P
                    nc.sync.dma_start(y[t0:t0 + P, :], ytok[:])

    nc.compile()
    return nc


def _prep_weights(inputs):
    """Host-side weight folding/packing. Returns dict of shared arrays."""
    g1, be1 = inputs["gamma1"], inputs["beta1"]
    g2, be2 = inputs["gamma2"], inputs["beta2"]
    Wqkv, bqkv = inputs["Wqkv"], inputs["bqkv"]
    Wo, bo = inputs["Wo"], inputs["bo"]
    W1, b1 = inputs["W1"], inputs["b1"]
    W2, b2 = inputs["W2"], inputs["b2"]
    gn, gb = inputs["gn"], inputs["gb"]
    Wdec, bdec = inputs["Wdec"], inputs["bdec"]

    wqkv_a = np.zeros((DEPTH, D, NH * 3 * P), np.float32)
    bqkv_a = np.zeros((DEPTH, NH * 3, P), np.float32)
    wo_a = np.zeros((DEPTH, NH * P, D), np.float32)
    bwo_a = np.zeros((DEPTH, KD, P), np.float32)
    w1_a = np.zeros((DEPTH, D, HID), np.float32)
    b1_a = np.zeros((DEPTH, HB, P), np.float32)
    w2_a = np.zeros((DEPTH, HID, D), np.float32)
    b2_a = np.zeros((DEPTH, KD, P), np.float32)
    scale = 1.0 / np.sqrt(DH)
    for l in range(DEPTH):
        Wp = Wqkv[l] * g1[l][None, :]                  # fold gamma1
        bp = bqkv[l] + Wqkv[l] @ be1[l]                # fold beta1
        Wp = Wp.copy()
        bp = bp.copy()
        Wp[:D] *= scale                                # fold 1/sqrt(dh) into Q
        bp[:D] *= scale
        for h in range(NH):
            for c in range(3):                         # q,k,v
                rows = slice(c * D + h * DH, c * D + (h + 1) * DH)
                wqkv_a[l, :, (h * 3 + c) * P:(h * 3 + c) * P + DH] = Wp[rows].T
                bqkv_a[l, h * 3 + c, :DH] = bp[rows]
            bqkv_a[l, h * 3 + 2, DH] = 1.0             # ones-row -> denominators
            wo_a[l, h * P:h * P + DH, :] = Wo[l][:, h * DH:(h + 1) * DH].T
        bwo_a[l] = bo[l].reshape(KD, P)
        w1_a[l] = (W1[l] * g2[l][None, :]).T
        b1_a[l] = (b1[l] + W1[l] @ be2[l]).reshape(HB, P)
        w2_a[l] = W2[l].T
        b2_a[l] = b2[l].reshape(KD, P)
    wdec_a = (Wdec * gn[None, :]).T
    bdec_a = (bdec + Wdec @ gb).reshape(KD, P)
    return {
        "wqkv": wqkv_a.astype(BF16), "bqkv": bqkv_a,
        "wo": wo_a.astype(BF16), "bwo": bwo_a,
        "w1": w1_a.astype(BF16), "b1": b1_a,
        "w2": w2_a.astype(BF16), "b2": b2_a,
        "wdec": wdec_a.astype(BF16), "bdec": bdec_a,
    }


def kernel(**inputs):
    from concourse.bass_utils import run_bass_kernel_spmd

    inputs = {k: np.asarray(v) for k, v in inputs.items()}
    if "nc" not in _cache:
        _cache["nc"] = _build()
    nc = _cache["nc"]

    shared = _prep_weights(inputs)
    mask = inputs["mask"]
    vt = inputs["visible_tokens"].astype(np.float32)
    mt = inputs["mask_token"].astype(np.float32)

    nv = np.clip(np.cumsum(mask.astype(np.int64), axis=1) - 1, 0, N_VIS - 1)
    idx_full = np.where(mask, nv, N_VIS).astype(np.int32)     # row 512 = mask token

    in_maps = []
    for core in range(8):
        b, s = core // 2, core % 2
        if s == 0:
            perm = np.concatenate([np.arange(TQ, T), np.arange(0, TQ)])
        else:
            perm = np.arange(T)
        vis_ext = np.concatenate([vt[b], mt[None, :]], axis=0)
        m = dict(shared)
        m["vis"] = np.ascontiguousarray(vis_ext)
        m["idx"] = np.ascontiguousarray(idx_full[b][perm][:, None])
        t_other = perm[:TQ].astype(np.int64)
        rows = (t_other // TQ) * CH + (t_other % CH)
        m["xidx"] = np.ascontiguousarray(rows.astype(np.int32)[:, None])
        in_maps.append(m)

    res = run_bass_kernel_spmd(nc, in_maps, core_ids=list(range(8)),
                               **_cache.get("run_kwargs", {}))
    _cache["last_results"] = res

    out = np.zeros((B, T, D), np.float32)
    for core in range(8):
        b, s = core // 2, core % 2
        out[b, s * TQ:(s + 1) * TQ] = res.results[core]["y"]
    return out


if __name__ == "__main__":
    rng = np.random.default_rng(0)
    print("building...")
    _build()
    print("built ok")
